# revision 44
# baseline (speedup 1.0000x reference)
"""Distributed Bass kernel for sliding-window GQA attention on 8 TRN2 NeuronCores.

Problem: B=2, S=2048, DIM=2048, H=16, KVH=4, HD=128, WINDOW=1024 (causal
sliding window), nonstandard RoPE producing 1.5*HD score features.

Sharding (tensor-parallel on the kv-head axis, data-parallel on batch —
no collectives): core c owns (batch, kv-group) = (c//4, c%4): its 4 q-heads
and 1 kv head over the full 2048-row sequence. wq/wk/wv are column-sharded
by kv group, wo row-sharded. Each core emits a PARTIAL output projection
(its 4 heads x its wo rows); the host sums the 4 partials per batch while
unsharding — replacing the all-reduce.

Structure: scores are computed TRANSPOSED (S^T[k, q], k on partitions) with
all 4 heads packed into one N=512 matmul pair per k-block — q1/k1 are
feature-major already so this is free. The imag-half (64-dim) contraction
is duplicated across both partition halves (wq imag columns pre-halved on
the host) so both score passes run K=128: a 64-row stationary gets a
row_grp-masked LDWEIGHTS that cannot overlap the in-flight matmul
(measured +210ns per pair). Sliding-window masking is a post-exp 0/1 bf16
multiply on the otherwise-idle GpSimd engine (replacing pre-exp -1e9 adds
on the congested Vector engine); masked PV groups are fired LAST from the
trailing window so the ~1.2us GpSimd latency is always covered. The
softmax row-sums come from a ones-column appended to V (PV out [q, 129]
carries the denominator in col 128), normalization happens during the
PSUM->SBUF attn copy (per-partition scalar mul), and a single 128x128
transpose matmul per (head, q-block) feeds the O-projection.

Pipelining: attention for q-blocks 4cq..4cq+3 is interleaved right after
column-chunk cq's projections; PV groups trail the score matmuls by 4
k-blocks and carry across q-block boundaries (so the exp on the Scalar
engine never stalls the in-order PE queue). The staged finish of q-block
qc advances one step per k-block inside qc+1's loop: norms (vector) lead
their transposes (PE) by a full step, and the transpose scratch comes from
the ps pool, not sps, so the finish chain never back-pressures the score
banks. Quarter 0 runs its q-blocks [3,2,1,0] so the 1-2-pair ramp blocks
sit against the quarter boundary where cq1's projections hide their
exp->mask->PV chains.

Prologue: the PE warm-up (HAM clock-gate lift) and exp-table prewarm run on
a memset tile with no DMA dependency, so they start as soon as the engines
initialize (~6.5us) instead of after the first weight DMA lands (~12us).
DMA issue order is strictly by first-use with few mid-sized issues (each
dma_start costs 0.6-1.2us of serial Sync time; concurrent transfers share
~400GB/s equally per outstanding issue): wk, x0 in 3 parts, wq halves,
fmp, wv, masks/identity, wo. Outputs go out as two fused bf16 DMAs per
q-block (four for the final q-block so the store overlaps the last
O-proj).
"""
import numpy as np
import ml_dtypes

import concourse.tile as tile
from concourse import bacc, mybir
from concourse.bass_utils import run_bass_kernel_spmd
from contextlib import ExitStack

F32 = mybir.dt.float32
BF16 = mybir.dt.bfloat16
EXP = mybir.ActivationFunctionType.Exp

B, S, DIM = 2, 2048, 2048
H, KVH, HD = 16, 4, 128
HPC = H // KVH  # heads per core (4)
WINDOW = 1024
SCALE = HD ** -0.5
NDC = DIM // 128  # 16 dim chunks
NQC = S // 128    # 16 q blocks

_cache = {}


def _kblocks(qc):
    return list(range(max(0, qc - 8), qc + 1))


def _build():
    nc = bacc.Bacc("TRN2", target_bir_lowering=False, debug=False, num_devices=8)

    xt_d = nc.dram_tensor("xt", [128, 4 * NDC * 512], BF16, kind="ExternalInput")
    wq_d = nc.dram_tensor("wq", [128, 2 * NDC * 256], BF16, kind="ExternalInput")
    wk_d = nc.dram_tensor("wk", [128, NDC * 128], BF16, kind="ExternalInput")
    wv_d = nc.dram_tensor("wv", [128, NDC * 128], BF16, kind="ExternalInput")
    wo_d = nc.dram_tensor("wo", [128, HPC * 2048], BF16, kind="ExternalInput")
    fmp_d = nc.dram_tensor("fmp", [128, S], BF16, kind="ExternalInput")
    msk_d = nc.dram_tensor("msk", [128, 1024], BF16, kind="ExternalInput")
    idn_d = nc.dram_tensor("idn", [128, 128], BF16, kind="ExternalInput")
    out_d = nc.dram_tensor("out", [S, DIM], BF16, kind="ExternalOutput")

    with tile.TileContext(nc) as tc, ExitStack() as ctx:
        xp = ctx.enter_context(tc.tile_pool(name="xp", bufs=3))
        wp = ctx.enter_context(tc.tile_pool(name="wp", bufs=1))
        cp = ctx.enter_context(tc.tile_pool(name="cp", bufs=1))
        qp = ctx.enter_context(tc.tile_pool(name="qp", bufs=1))
        kp = ctx.enter_context(tc.tile_pool(name="kp", bufs=1))
        vp = ctx.enter_context(tc.tile_pool(name="vp", bufs=1))
        pp = ctx.enter_context(tc.tile_pool(name="pp", bufs=12))
        ap_ = ctx.enter_context(tc.tile_pool(name="ap", bufs=8))
        atp = ctx.enter_context(tc.tile_pool(name="atp", bufs=2))
        rp = ctx.enter_context(tc.tile_pool(name="rp", bufs=8))
        op_ = ctx.enter_context(tc.tile_pool(name="op", bufs=2))
        # PSUM: 8 banks = ps(2: proj + O-proj) + sps(3: scores + attn
        # transposes) + pvs(3: PV accumulators, 2 heads per bank)
        ps = ctx.enter_context(tc.tile_pool(name="ps", bufs=2, space="PSUM"))
        sps = ctx.enter_context(tc.tile_pool(name="sps", bufs=3, space="PSUM"))
        pvs = ctx.enter_context(tc.tile_pool(name="pvs", bufs=3, space="PSUM"))

        # ---- persistent SBUF tensors ----
        # k2/q2 duplicate the imag half across both partition halves (wq imag
        # columns pre-halved on the host): a 64-row stationary would get a
        # row_grp-masked LDWEIGHTS that cannot overlap the in-flight matmul
        # (measured +210ns per score pair), so both passes stay K=128.
        q1 = qp.tile([128, NQC, 512], BF16, tag="q1")  # [feat, qblock, h*128+q]
        q2 = qp.tile([128, NQC, 512], BF16, tag="q2")  # imag, duplicated halves
        k1 = kp.tile([128, S], BF16, tag="k1")
        k2 = kp.tile([128, S], BF16, tag="k2")  # imag, duplicated halves
        v_sb = vp.tile([128, NQC, 132], BF16, tag="v")  # col 128 = ones

        # warm-up source with no DMA dependency: PE warm-up matmuls lift the
        # HAM clock gate and the first activation pulls the exp table-set
        # (~2.7us) while the first input DMAs are still in flight
        wz = cp.tile([128, 512], BF16, tag="wz")
        nc.vector.memset(wz[:], 0.0)
        warm = rp.tile([128, 1], F32, tag="rc", name="warm")
        nc.scalar.activation(warm[:], wz[:, 0:1], EXP)
        # 12 N=512 matmuls = ~5.1us of sustained PE activity at the cold
        # rate: covers 1.5 free-running HAM SHORT windows so the clock gate
        # reliably lifts to 2.4 GHz before the Q-projection (an 8-matmul
        # burst spans just one window and missed the flip on most runs,
        # leaving projections at half rate until ~20us). Costs nothing: the
        # K-projection is DMA-paced until past this point anyway.
        wups = ps.tile([128, 512], F32, tag="ps", name="wups")
        for i in range(12):
            nc.tensor.matmul(
                wups[:], wz[:, 0:128], wz[:], start=(i == 0), stop=(i == 11),
            )

        # ---- DMA issue order = transfer order: strictly by first use ----
        wk_t = wp.tile([128, NDC, 128], BF16, tag="wk")
        wv_t = wp.tile([128, NDC, 128], BF16, tag="wv")

        def load_x(cq, parts):
            x_q = xp.tile([128, NDC, 512], BF16, tag="x", name=f"x{cq}")
            dg = 0
            for w_dg in parts:
                nc.sync.dma_start(
                    x_q[:, dg : dg + w_dg, :],
                    xt_d[
                        :,
                        cq * NDC * 512 + dg * 512 : cq * NDC * 512
                        + (dg + w_dg) * 512,
                    ],
                )
                dg += w_dg
            return x_q

        # front DMA order = first-use order. Few, mid-sized issues: each
        # dma_start costs 0.6-1.2us of serial Sync time, and concurrent
        # transfers share bandwidth round-robin, so a long tail of small
        # parts delays everything behind it (measured x0 landing at 20us)
        # transfers share bandwidth equally per outstanding dma_start, so the
        # split also acts as a priority: x0 keeps 2-3 queues against wq's one
        nc.sync.dma_start(wk_t[:], wk_d[:, :])
        x_q0 = xp.tile([128, NDC, 512], BF16, tag="x", name="x0")
        wq_t = [
            wp.tile([128, NDC, 256], BF16, tag=f"wq{i}", name=f"wq{i}")
            for i in range(2)
        ]
        nc.sync.dma_start(x_q0[:, 0:2, :], xt_d[:, 0:1024])
        nc.sync.dma_start(x_q0[:, 2:8, :], xt_d[:, 1024:4096])
        nc.sync.dma_start(x_q0[:, 8:16, :], xt_d[:, 4096:8192])
        x_tiles = {0: x_q0}

        fmp = cp.tile([128, S], BF16, tag="fmp")  # fm rows 0:64, fp 64:128
        msk = cp.tile([128, 1024], BF16, tag="msk")  # [tail 0:512 | diag 512:1024]
        idn = cp.tile([128, 128], BF16, tag="idn")
        wo_t = None
        pending = None  # (pv tiles, qc) awaiting normalize/transpose/O-proj
        window = []  # trailing (pv, kb, p_sb, masked) PV groups, oldest first

        def pv_group(pv, kb, p_sb, is_stop):
            for h in range(HPC):
                nc.tensor.matmul(
                    pv[h // 2][:, (h % 2) * 132 : (h % 2) * 132 + 129],
                    p_sb[:, h * 128 : (h + 1) * 128],
                    v_sb[:, kb, 0:129],
                    start=False,
                    stop=is_stop,
                )

        def fire(idx):
            pv, kb, p_sb, masked = window.pop(idx)
            is_stop = not any(w[0] is pv for w in window)
            pv_group(pv, kb, p_sb, is_stop)

        def fire_one():
            # prefer the oldest UNMASKED group: masked p tiles wait on the
            # GpSimd mask multiply (~1.2us), so they fire as late as
            # possible — accumulation order into the PV bank is free
            for j, w in enumerate(window):
                if not w[3]:
                    fire(j)
                    return
            fire(0)

        def flush_to(pv_tiles):
            # a finish reads its PV accumulators: every trailing group that
            # targets them must be emitted first (masked ones last)
            while True:
                idxs = [j for j, w in enumerate(window) if w[0] is pv_tiles]
                if not idxs:
                    return
                unm = [j for j in idxs if not window[j][3]]
                fire(unm[0] if unm else idxs[0])

        def finish_norm(st, h):
            # head h's normalize on the vector engine
            pv = st["pv"]
            c0 = (h % 2) * 132
            if h % 2 == 0:
                # both heads' softmax denominators sit at cols 128/260 of the
                # shared bank: one strided reciprocal covers the pair
                rc = rp.tile([128, 2], F32, tag="rc")
                st["rc"] = rc
                nc.vector.reciprocal(rc[:], pv[h // 2][:, 128:261:132])
            rc = st["rc"]
            a_sb = ap_.tile([128, 128], BF16, tag="a")
            st["a"][h] = a_sb
            nc.vector.tensor_scalar_mul(
                a_sb[:], pv[h // 2][:, c0 : c0 + 128], rc[:, h % 2 : h % 2 + 1]
            )

        def finish_trans(st, h, tpool):
            # head h's transpose, one step AFTER its norm: the PE-side
            # transpose otherwise waits in-step on the vector norm chain
            # (measured ~670ns stall per head)
            if st["at"] is None:
                st["at"] = atp.tile([128, HPC, 128], BF16, tag="at", name="at_sb")
            at_sb = st["at"]
            # transpose scratch comes from whichever PSUM pool is idle in the
            # calling context (ps during attention — an sps tile would couple
            # the finish into the score-pair banks; sps during the projection
            # phase — a ps tile would halve the projection-group ring)
            tps = tpool.tile([128, 512], F32, tag="ps" if tpool is ps else "s",
                             name=f"tp{h}")
            nc.tensor.matmul(tps[:, 0:128], st["a"][h], idn, start=True, stop=True)
            # all 4 copies on vector: scalar is the busier engine during
            # attention (one 690ns exp per pair) and queued copies there
            # delay exps, which shows up as score-matmul stalls
            nc.vector.tensor_copy(at_sb[:, h, :], tps[:, 0:128])

        def finish_tail(st):
            at_sb = st["at"]
            qb = st["qc"] * 128
            last = st["qc"] == NQC - 1
            o_sb = op_.tile([128, 2048], BF16, tag="o")
            for dn in range(4):
                ops = ps.tile([128, 512], F32, tag="ps")
                for f in range(HPC):
                    nc.tensor.matmul(
                        ops[:],
                        at_sb[:, f, :],
                        wo_t[:, f, dn * 512 : (dn + 1) * 512],
                        start=(f == 0),
                        stop=(f == HPC - 1),
                    )
                nc.vector.tensor_copy(o_sb[:, dn * 512 : (dn + 1) * 512], ops[:])
                if last:
                    # final q-block: store each quarter as soon as its copy
                    # lands so the DMA overlaps the remaining O-proj matmuls
                    nc.sync.dma_start(
                        out_d[qb : qb + 128, dn * 512 : (dn + 1) * 512],
                        o_sb[:, dn * 512 : (dn + 1) * 512],
                    )
                elif dn == 1:
                    nc.sync.dma_start(
                        out_d[qb : qb + 128, 0:1024], o_sb[:, 0:1024]
                    )
            # two DMAs per q-block: fused enough to keep Sync sequencing
            # cheap (~600ns per dma_start), split so the final q-block's
            # store overlaps its second O-proj half
            if not last:
                nc.sync.dma_start(
                    out_d[qb : qb + 128, 1024:2048], o_sb[:, 1024:2048]
                )

        def finish_step(st, tpool=None):
            # advance the staged finish by one unit; True when fully done.
            # Stages: 0: flush+norm0, 1-3: norm h / transpose h-1, 4:
            # transpose 3, 5: O-proj tail
            s = st["step"]
            if s == 0:
                flush_to(st["pv"])
                finish_norm(st, 0)
            elif s <= 3:
                finish_norm(st, s)
                finish_trans(st, s - 1, ps if tpool is None else tpool)
            elif s == 4:
                finish_trans(st, 3, ps if tpool is None else tpool)
            else:
                finish_tail(st)
            st["step"] += 1
            return st["step"] > HPC + 1

        def emit_finish(st):
            while not finish_step(st):
                pass

        for cq in range(4):
            # ---- projections for column-quarter cq ----
            x_q = x_tiles.pop(cq)
            if cq == 0:
                # remaining weights/constants in first-use order; wv/wo land
                # during this quarter's Q-proj
                nc.sync.dma_start(wq_t[0][:], wq_d[:, 0:4096])
                nc.sync.dma_start(fmp[:, 0:512], fmp_d[:, 0:512])
                nc.sync.dma_start(wq_t[1][:], wq_d[:, 4096:8192])
                nc.sync.dma_start(wv_t[:], wv_d[:, :])
                nc.sync.dma_start(msk[:], msk_d[:, :])
                nc.sync.dma_start(idn[:], idn_d[:, :])
                nc.vector.memset(v_sb[:, :, 128:129], 1.0)
                wo_t = wp.tile([128, HPC, 2048], BF16, tag="wo")
                for i in range(2):
                    nc.sync.dma_start(
                        wo_t[:, i * 2 : (i + 1) * 2, :],
                        wo_d[:, i * 4096 : (i + 1) * 4096],
                    )
                nc.sync.dma_start(fmp[:, 512:S], fmp_d[:, 512:S])
            cs = slice(cq * 512, (cq + 1) * 512)
            fmc, fpc = fmp[0:64, cs], fmp[64:128, cs]
            blk = slice(cq * 4, (cq + 1) * 4)

            def q_rope(h, qps):
                hc = slice(h * 128, (h + 1) * 128)
                src = qps[0:64, :].rearrange("p (b q) -> p b q", b=4)
                nc.vector.tensor_mul(
                    q1[0:64, blk, hc], src, fmc.rearrange("p (b q) -> p b q", b=4)
                )
                nc.vector.tensor_mul(
                    q1[64:128, blk, hc], src, fpc.rearrange("p (b q) -> p b q", b=4)
                )
                nc.scalar.copy(
                    q2[0:64, blk, hc],
                    qps[64:128, :].rearrange("p (b q) -> p b q", b=4),
                )
                nc.scalar.copy(
                    q2[64:128, blk, hc],
                    qps[64:128, :].rearrange("p (b q) -> p b q", b=4),
                )

            # K projection + rope
            kps = ps.tile([128, 512], F32, tag="ps")
            for dc in range(NDC):
                nc.tensor.matmul(
                    kps[:],
                    wk_t[:, dc, :],
                    x_q[:, dc, :],
                    start=(dc == 0),
                    stop=(dc == NDC - 1),
                )
            nc.vector.tensor_mul(k1[0:64, cs], kps[0:64, :], fmc)
            nc.vector.tensor_mul(k1[64:128, cs], kps[0:64, :], fpc)
            nc.scalar.copy(k2[0:64, cs], kps[64:128, :])
            nc.scalar.copy(k2[64:128, cs], kps[64:128, :])
            # the previous quarter's last finish advances one stage per
            # projection group: its recip/norm vector chain hides under the
            # 3.4us matmul streams instead of stalling the proj->attention
            # seam (transposes draw sps scratch — idle during projections)
            if pending is not None and finish_step(pending, sps):
                pending = None

            # Q projections + rope, packed layout [feat, qblock, h*128+q]
            for h in range(HPC):
                qps = ps.tile([128, 512], F32, tag="ps")
                for dc in range(NDC):
                    nc.tensor.matmul(
                        qps[:],
                        wq_t[h // 2][:, dc, (h % 2) * 128 : (h % 2 + 1) * 128],
                        x_q[:, dc, :],
                        start=(dc == 0),
                        stop=(dc == NDC - 1),
                    )
                q_rope(h, qps)
                if pending is not None and finish_step(pending, sps):
                    pending = None

            # V projection -> v_sb [kpos, feat] (first consumed mid-way into
            # this quarter's attention, so it sits after Q to shorten the
            # rope -> first-scores critical chain)
            vps = ps.tile([128, 512], F32, tag="ps")
            for kb4 in range(4):
                for dc in range(NDC):
                    nc.tensor.matmul(
                        vps[:, kb4 * 128 : (kb4 + 1) * 128],
                        x_q[:, dc, kb4 * 128 : (kb4 + 1) * 128],
                        wv_t[:, dc, :],
                        start=(dc == 0),
                        stop=(dc == NDC - 1),
                    )
            nc.vector.tensor_copy(
                v_sb[:, cq * 4 : (cq + 1) * 4, 0:128],
                vps[:].rearrange("p (b q) -> p b q", b=4),
            )

            # prefetch next x chunk during this quarter's attention
            if cq + 1 < 4:
                x_tiles[cq + 1] = load_x(cq + 1, [8, 8])

            # finish the previous quarter's last q-block now: its transposes
            # + O-projection give the PE work while the rope tail (vector/
            # scalar) of this quarter completes
            if pending is not None:
                emit_finish(pending)
                pending = None

            # ---- attention for q-blocks of this quarter ----
            # quarter 0 runs [3,2,1,0]: the ramp q-blocks (1-2 score pairs)
            # can't hide their exp->mask->PV chains, so they sit against the
            # quarter boundary where cq1's projections cover them
            qcs = [3, 2, 1, 0] if cq == 0 else range(cq * 4, (cq + 1) * 4)
            for qc in qcs:
                kbs = _kblocks(qc)
                nkb = len(kbs)
                pv = [
                    pvs.tile([128, 264], F32, tag="pv", name=f"pv{qc}_{i}")
                    for i in range(2)
                ]
                # Two heads accumulate in one bank: a start=True matmul would
                # clear the co-resident head's has_written bits mid-group, so
                # zero the bank and accumulate with start=False throughout
                # (add-where-set on zeros / overwrite-where-clear both work).
                for t in pv:
                    nc.vector.memset(t[:], 0.0)

                # masked k-blocks (diagonal, window tail) first: their extra
                # post-exp mask multiply sits on the exp->PV chain, so keep
                # them off the q-block tail where that chain is exposed
                kbs_proc = [kbs[-1]] + ([kbs[0]] if nkb > 1 else []) + kbs[1:-1]
                for mi, kb in enumerate(kbs_proc):
                    sp = sps.tile([128, 512], F32, tag="s")
                    lo = kb * 128
                    nc.tensor.matmul(
                        sp[:], k1[:, lo : lo + 128], q1[:, qc, :],
                        start=True, stop=False,
                    )
                    nc.tensor.matmul(
                        sp[:], k2[:, lo : lo + 128], q2[:, qc, :],
                        start=False, stop=True,
                    )
                    p_sb = pp.tile([128, 512], BF16, tag="p")
                    nc.scalar.activation(p_sb[:], sp[:], EXP)
                    # sliding-window mask: 0/1 multiply after the exp, on the
                    # otherwise-idle GpSimd engine (SBUF-only op)
                    masked = kb == qc or kb == qc - 8
                    if kb == qc:
                        nc.gpsimd.tensor_mul(p_sb[:], p_sb[:], msk[:, 512:1024])
                    if kb == qc - 8:
                        nc.gpsimd.tensor_mul(p_sb[:], p_sb[:], msk[:, 0:512])
                    # the PV group trails by 4 k-blocks, carried across
                    # q-block boundaries so score matmuls always cover the
                    # exp latency
                    window.append((pv, kb, p_sb, masked))
                    # the previous q-block's finish advances one stage per
                    # k-block: each transpose trails the vector recip/norm
                    # chain with a score pair of PE cover in between
                    if pending is not None and mi >= 1:
                        if finish_step(pending):
                            pending = None
                    while len(window) > 4:
                        fire_one()
                while pending is not None:
                    if finish_step(pending):
                        pending = None
                pending = {
                    "pv": pv, "qc": qc, "at": None, "rc": None, "a": {},
                    "step": 0,
                }

        emit_finish(pending)

    nc.compile()
    return nc


def _prep_core(inputs, c):
    x = inputs["x"]
    cos, sin = np.asarray(inputs["cos"]), np.asarray(inputs["sin"])
    mask = np.asarray(inputs["mask"])
    wq = np.asarray(inputs["wq"], dtype=np.float32)
    wk = np.asarray(inputs["wk"], dtype=np.float32)
    wv = np.asarray(inputs["wv"], dtype=np.float32)
    wo = np.asarray(inputs["wo"], dtype=np.float32)
    bf = ml_dtypes.bfloat16
    b, g = c // 4, c % 4

    # x[b] transposed -> [128p, cq, dc, 512]
    xt = np.asarray(x[b], dtype=np.float32).T  # [dim, S]
    xt = xt.reshape(NDC, 128, 4, 512).transpose(1, 2, 0, 3)
    xt = np.ascontiguousarray(xt).reshape(128, 4 * NDC * 512).astype(bf)

    # wq slice for heads 4g..4g+3 (SCALE folded). The imag half-columns are
    # additionally halved: the kernel duplicates q2/k2 across both partition
    # halves, doubling the imag contraction.
    wqs = (wq[:, g * 512 : (g + 1) * 512] * SCALE).reshape(DIM, HPC, 128).copy()
    wqs[:, :, 64:128] *= 0.5
    wqs = wqs.reshape(NDC, 128, 2, 256)
    wqs = np.ascontiguousarray(wqs.transpose(1, 2, 0, 3)).reshape(128, 2 * NDC * 256)
    # wk / wv slices for kv head g: [p, dc, 128]
    wks = wk[:, g * 128 : (g + 1) * 128].reshape(NDC, 128, 128).transpose(1, 0, 2)
    wks = np.ascontiguousarray(wks).reshape(128, NDC * 128)
    wvs = wv[:, g * 128 : (g + 1) * 128].reshape(NDC, 128, 128).transpose(1, 0, 2)
    wvs = np.ascontiguousarray(wvs).reshape(128, NDC * 128)
    # wo rows for this core's heads: [p, h, 2048] tiles
    wos = wo[g * 512 : (g + 1) * 512].reshape(HPC, 128, 2048).transpose(1, 0, 2)
    wos = np.ascontiguousarray(wos).reshape(128, HPC * 2048)

    # fm rows 0:64, fp rows 64:128 (one tensor -> one DMA)
    fmp = np.concatenate([(cos - sin).T, (cos + sin).T], axis=0)
    fmp = np.ascontiguousarray(fmp, dtype=np.float32).astype(bf)
    # 0/1 keep-masks, transposed for the S^T layout, tiled across the 4
    # packed heads: [tail block | diagonal block]
    tail01 = (mask[WINDOW : WINDOW + 128, 0:128].T == 0.0).astype(np.float32)
    diag01 = (mask[0:128, 0:128].T == 0.0).astype(np.float32)
    msk = np.concatenate([np.tile(tail01, (1, 4)), np.tile(diag01, (1, 4))], axis=1)
    msk = np.ascontiguousarray(msk).astype(bf)
    idn = np.ascontiguousarray(np.eye(128, dtype=np.float32)).astype(bf)

    return {
        "xt": xt, "wq": wqs.astype(bf), "wk": wks.astype(bf), "wv": wvs.astype(bf),
        "wo": wos.astype(bf), "fmp": fmp, "msk": msk, "idn": idn,
    }


def kernel(**inputs) -> np.ndarray:
    if "nc" not in _cache:
        _cache["nc"] = _build()
    nc = _cache["nc"]
    in_maps = [_prep_core(inputs, c) for c in range(8)]
    res = run_bass_kernel_spmd(nc, in_maps, core_ids=list(range(8)))
    out = np.zeros((B, S, DIM), dtype=np.float32)
    for c in range(8):
        out[c // 4] += np.asarray(res.results[c]["out"], dtype=np.float32)
    return out


# revision 46
# speedup vs baseline: 1.0037x; 1.0037x over previous
"""Distributed Bass kernel for sliding-window GQA attention on 8 TRN2 NeuronCores.

Problem: B=2, S=2048, DIM=2048, H=16, KVH=4, HD=128, WINDOW=1024 (causal
sliding window), nonstandard RoPE producing 1.5*HD score features.

Sharding (tensor-parallel on the kv-head axis, data-parallel on batch —
no collectives): core c owns (batch, kv-group) = (c//4, c%4): its 4 q-heads
and 1 kv head over the full 2048-row sequence. wq/wk/wv are column-sharded
by kv group, wo row-sharded. Each core emits a PARTIAL output projection
(its 4 heads x its wo rows); the host sums the 4 partials per batch while
unsharding — replacing the all-reduce.

Structure: scores are computed TRANSPOSED (S^T[k, q], k on partitions) with
all 4 heads packed into one N=512 matmul pair per k-block — q1/k1 are
feature-major already so this is free. The imag-half (64-dim) contraction
is duplicated across both partition halves (wq imag columns pre-halved on
the host) so both score passes run K=128: a 64-row stationary gets a
row_grp-masked LDWEIGHTS that cannot overlap the in-flight matmul
(measured +210ns per pair). Sliding-window masking is a post-exp 0/1 bf16
multiply on the otherwise-idle GpSimd engine (replacing pre-exp -1e9 adds
on the congested Vector engine); masked PV groups are fired LAST from the
trailing window so the ~1.2us GpSimd latency is always covered. The
softmax row-sums come from a ones-column appended to V (PV out [q, 129]
carries the denominator in col 128), normalization happens during the
PSUM->SBUF attn copy (per-partition scalar mul), and a single 128x128
transpose matmul per (head, q-block) feeds the O-projection.

Pipelining: attention for q-blocks 4cq..4cq+3 is interleaved right after
column-chunk cq's projections; PV groups trail the score matmuls by 4
k-blocks and carry across q-block boundaries (so the exp on the Scalar
engine never stalls the in-order PE queue). The staged finish of q-block
qc advances one step per k-block inside qc+1's loop: norms (vector) lead
their transposes (PE) by a full step, and the transpose scratch comes from
the ps pool, not sps, so the finish chain never back-pressures the score
banks. Quarter 0 runs its q-blocks [3,2,1,0] so the 1-2-pair ramp blocks
sit against the quarter boundary where cq1's projections hide their
exp->mask->PV chains.

Prologue: the PE warm-up (HAM clock-gate lift) and exp-table prewarm run on
a memset tile with no DMA dependency, so they start as soon as the engines
initialize (~6.5us) instead of after the first weight DMA lands (~12us).
DMA issue order is strictly by first-use with few mid-sized issues (each
dma_start costs 0.6-1.2us of serial Sync time; concurrent transfers share
~400GB/s equally per outstanding issue): wk, x0 in 3 parts, wq halves,
fmp, wv, masks/identity, wo. Outputs go out as two fused bf16 DMAs per
q-block (four for the final q-block so the store overlaps the last
O-proj).
"""
import numpy as np
import ml_dtypes

import concourse.tile as tile
from concourse import bacc, mybir
from concourse.bass_utils import run_bass_kernel_spmd
from contextlib import ExitStack

F32 = mybir.dt.float32
BF16 = mybir.dt.bfloat16
EXP = mybir.ActivationFunctionType.Exp

B, S, DIM = 2, 2048, 2048
H, KVH, HD = 16, 4, 128
HPC = H // KVH  # heads per core (4)
WINDOW = 1024
SCALE = HD ** -0.5
NDC = DIM // 128  # 16 dim chunks
NQC = S // 128    # 16 q blocks

_cache = {}


def _kblocks(qc):
    return list(range(max(0, qc - 8), qc + 1))


def _build():
    nc = bacc.Bacc("TRN2", target_bir_lowering=False, debug=False, num_devices=8)

    xt_d = nc.dram_tensor("xt", [128, 4 * NDC * 512], BF16, kind="ExternalInput")
    wq_d = nc.dram_tensor("wq", [128, 2 * NDC * 256], BF16, kind="ExternalInput")
    wk_d = nc.dram_tensor("wk", [128, NDC * 128], BF16, kind="ExternalInput")
    wv_d = nc.dram_tensor("wv", [128, NDC * 128], BF16, kind="ExternalInput")
    wo_d = nc.dram_tensor("wo", [128, HPC * 2048], BF16, kind="ExternalInput")
    fmp_d = nc.dram_tensor("fmp", [128, S], BF16, kind="ExternalInput")
    msk_d = nc.dram_tensor("msk", [128, 1024], BF16, kind="ExternalInput")
    idn_d = nc.dram_tensor("idn", [128, 128], BF16, kind="ExternalInput")
    out_d = nc.dram_tensor("out", [S, DIM], BF16, kind="ExternalOutput")

    with tile.TileContext(nc) as tc, ExitStack() as ctx:
        xp = ctx.enter_context(tc.tile_pool(name="xp", bufs=3))
        wp = ctx.enter_context(tc.tile_pool(name="wp", bufs=1))
        cp = ctx.enter_context(tc.tile_pool(name="cp", bufs=1))
        qp = ctx.enter_context(tc.tile_pool(name="qp", bufs=1))
        kp = ctx.enter_context(tc.tile_pool(name="kp", bufs=1))
        vp = ctx.enter_context(tc.tile_pool(name="vp", bufs=1))
        pp = ctx.enter_context(tc.tile_pool(name="pp", bufs=12))
        ap_ = ctx.enter_context(tc.tile_pool(name="ap", bufs=8))
        atp = ctx.enter_context(tc.tile_pool(name="atp", bufs=2))
        rp = ctx.enter_context(tc.tile_pool(name="rp", bufs=8))
        op_ = ctx.enter_context(tc.tile_pool(name="op", bufs=2))
        # PSUM: 8 banks = ps(2: proj + O-proj) + sps(3: scores + attn
        # transposes) + pvs(3: PV accumulators, 2 heads per bank)
        ps = ctx.enter_context(tc.tile_pool(name="ps", bufs=2, space="PSUM"))
        sps = ctx.enter_context(tc.tile_pool(name="sps", bufs=3, space="PSUM"))
        pvs = ctx.enter_context(tc.tile_pool(name="pvs", bufs=3, space="PSUM"))

        # ---- persistent SBUF tensors ----
        # k2/q2 duplicate the imag half across both partition halves (wq imag
        # columns pre-halved on the host): a 64-row stationary would get a
        # row_grp-masked LDWEIGHTS that cannot overlap the in-flight matmul
        # (measured +210ns per score pair), so both passes stay K=128.
        q1 = qp.tile([128, NQC, 512], BF16, tag="q1")  # [feat, qblock, h*128+q]
        q2 = qp.tile([128, NQC, 512], BF16, tag="q2")  # imag, duplicated halves
        k1 = kp.tile([128, S], BF16, tag="k1")
        k2 = kp.tile([128, S], BF16, tag="k2")  # imag, duplicated halves
        v_sb = vp.tile([128, NQC, 132], BF16, tag="v")  # col 128 = ones

        # warm-up source with no DMA dependency: PE warm-up matmuls lift the
        # HAM clock gate and the first activation pulls the exp table-set
        # (~2.7us) while the first input DMAs are still in flight
        wz = cp.tile([128, 512], BF16, tag="wz")
        nc.vector.memset(wz[:], 0.0)
        warm = rp.tile([128, 1], F32, tag="rc", name="warm")
        nc.scalar.activation(warm[:], wz[:, 0:1], EXP)
        # 12 N=512 matmuls = ~5.1us of sustained PE activity at the cold
        # rate: covers 1.5 free-running HAM SHORT windows so the clock gate
        # reliably lifts to 2.4 GHz before the Q-projection (an 8-matmul
        # burst spans just one window and missed the flip on most runs,
        # leaving projections at half rate until ~20us). Costs nothing: the
        # K-projection is DMA-paced until past this point anyway.
        wups = ps.tile([128, 512], F32, tag="ps", name="wups")
        for i in range(12):
            nc.tensor.matmul(
                wups[:], wz[:, 0:128], wz[:], start=(i == 0), stop=(i == 11),
            )

        # ---- DMA issue order = transfer order: strictly by first use ----
        wk_t = wp.tile([128, NDC, 128], BF16, tag="wk")
        wv_t = wp.tile([128, NDC, 128], BF16, tag="wv")

        def load_x(cq, parts):
            x_q = xp.tile([128, NDC, 512], BF16, tag="x", name=f"x{cq}")
            dg = 0
            for w_dg in parts:
                nc.sync.dma_start(
                    x_q[:, dg : dg + w_dg, :],
                    xt_d[
                        :,
                        cq * NDC * 512 + dg * 512 : cq * NDC * 512
                        + (dg + w_dg) * 512,
                    ],
                )
                dg += w_dg
            return x_q

        # front DMA order = first-use order. Few, mid-sized issues: each
        # dma_start costs 0.6-1.2us of serial Sync time, and concurrent
        # transfers share bandwidth round-robin, so a long tail of small
        # parts delays everything behind it (measured x0 landing at 20us)
        # transfers share bandwidth equally per outstanding dma_start, so the
        # split also acts as a priority: x0 keeps 2-3 queues against wq's one
        nc.sync.dma_start(wk_t[:], wk_d[:, :])
        x_q0 = xp.tile([128, NDC, 512], BF16, tag="x", name="x0")
        wq_t = [
            wp.tile([128, NDC, 256], BF16, tag=f"wq{i}", name=f"wq{i}")
            for i in range(2)
        ]
        nc.sync.dma_start(x_q0[:, 0:2, :], xt_d[:, 0:1024])
        nc.sync.dma_start(x_q0[:, 2:8, :], xt_d[:, 1024:4096])
        nc.sync.dma_start(x_q0[:, 8:16, :], xt_d[:, 4096:8192])
        x_tiles = {0: x_q0}

        fmp = cp.tile([128, S], BF16, tag="fmp")  # fm rows 0:64, fp 64:128
        msk = cp.tile([128, 1024], BF16, tag="msk")  # [tail 0:512 | diag 512:1024]
        idn = cp.tile([128, 128], BF16, tag="idn")
        wo_t = None
        pending = None  # (pv tiles, qc) awaiting normalize/transpose/O-proj
        window = []  # trailing (pv, kb, p_sb, masked) PV groups, oldest first

        def pv_group(pv, kb, p_sb, is_stop):
            for h in range(HPC):
                nc.tensor.matmul(
                    pv[h // 2][:, (h % 2) * 132 : (h % 2) * 132 + 129],
                    p_sb[:, h * 128 : (h + 1) * 128],
                    v_sb[:, kb, 0:129],
                    start=False,
                    stop=is_stop,
                )

        def fire(idx):
            pv, kb, p_sb, masked = window.pop(idx)
            is_stop = not any(w[0] is pv for w in window)
            pv_group(pv, kb, p_sb, is_stop)

        def fire_one():
            # prefer the oldest UNMASKED group: masked p tiles wait on the
            # GpSimd mask multiply (~1.2us), so they fire as late as
            # possible — accumulation order into the PV bank is free
            for j, w in enumerate(window):
                if not w[3]:
                    fire(j)
                    return
            fire(0)

        def flush_to(pv_tiles):
            # a finish reads its PV accumulators: every trailing group that
            # targets them must be emitted first (masked ones last)
            while True:
                idxs = [j for j, w in enumerate(window) if w[0] is pv_tiles]
                if not idxs:
                    return
                unm = [j for j in idxs if not window[j][3]]
                fire(unm[0] if unm else idxs[0])

        def finish_norm(st, h):
            # head h's normalize on the vector engine
            pv = st["pv"]
            c0 = (h % 2) * 132
            if h % 2 == 0:
                # both heads' softmax denominators sit at cols 128/260 of the
                # shared bank: one strided reciprocal covers the pair
                rc = rp.tile([128, 2], F32, tag="rc")
                st["rc"] = rc
                nc.vector.reciprocal(rc[:], pv[h // 2][:, 128:261:132])
            rc = st["rc"]
            a_sb = ap_.tile([128, 128], BF16, tag="a")
            st["a"][h] = a_sb
            nc.vector.tensor_scalar_mul(
                a_sb[:], pv[h // 2][:, c0 : c0 + 128], rc[:, h % 2 : h % 2 + 1]
            )

        def finish_trans(st, h):
            # head h's transpose, one step AFTER its norm: the PE-side
            # transpose otherwise waits in-step on the vector norm chain
            # (measured ~670ns stall per head)
            if st["at"] is None:
                st["at"] = atp.tile([128, HPC, 128], BF16, tag="at", name="at_sb")
            at_sb = st["at"]
            # transpose scratch comes from the ps pool (idle until the O-proj
            # step), NOT sps: an sps tile would couple the finish chain into
            # the score-pair banks via WAR deps
            tps = ps.tile([128, 512], F32, tag="ps", name=f"tp{h}")
            nc.tensor.matmul(tps[:, 0:128], st["a"][h], idn, start=True, stop=True)
            # all 4 copies on vector: scalar is the busier engine during
            # attention (one 690ns exp per pair) and queued copies there
            # delay exps, which shows up as score-matmul stalls
            nc.vector.tensor_copy(at_sb[:, h, :], tps[:, 0:128])

        def finish_tail(st):
            at_sb = st["at"]
            qb = st["qc"] * 128
            last = st["qc"] == NQC - 1
            o_sb = op_.tile([128, 2048], BF16, tag="o")
            for dn in range(4):
                ops = ps.tile([128, 512], F32, tag="ps")
                for f in range(HPC):
                    nc.tensor.matmul(
                        ops[:],
                        at_sb[:, f, :],
                        wo_t[:, f, dn * 512 : (dn + 1) * 512],
                        start=(f == 0),
                        stop=(f == HPC - 1),
                    )
                # copies drain two-wide across vector+scalar: four serialized
                # vector CASTs (690ns each) otherwise delay the output DMA
                if dn % 2 == 0:
                    nc.vector.tensor_copy(
                        o_sb[:, dn * 512 : (dn + 1) * 512], ops[:]
                    )
                else:
                    nc.scalar.copy(o_sb[:, dn * 512 : (dn + 1) * 512], ops[:])
                if last:
                    # final q-block: store each quarter as soon as its copy
                    # lands so the DMA overlaps the remaining O-proj matmuls
                    nc.sync.dma_start(
                        out_d[qb : qb + 128, dn * 512 : (dn + 1) * 512],
                        o_sb[:, dn * 512 : (dn + 1) * 512],
                    )
                elif dn == 1:
                    nc.sync.dma_start(
                        out_d[qb : qb + 128, 0:1024], o_sb[:, 0:1024]
                    )
            # two DMAs per q-block: fused enough to keep Sync sequencing
            # cheap (~600ns per dma_start), split so the final q-block's
            # store overlaps its second O-proj half
            if not last:
                nc.sync.dma_start(
                    out_d[qb : qb + 128, 1024:2048], o_sb[:, 1024:2048]
                )

        def finish_step(st):
            # advance the staged finish by one unit; True when fully done.
            # Stages: 0: flush+norm0, 1-3: norm h / transpose h-1, 4:
            # transpose 3, 5: O-proj tail
            s = st["step"]
            if s == 0:
                flush_to(st["pv"])
                finish_norm(st, 0)
            elif s <= 3:
                finish_norm(st, s)
                finish_trans(st, s - 1)
            elif s == 4:
                finish_trans(st, 3)
            else:
                finish_tail(st)
            st["step"] += 1
            return st["step"] > HPC + 1

        def emit_finish(st):
            while not finish_step(st):
                pass

        for cq in range(4):
            # ---- projections for column-quarter cq ----
            x_q = x_tiles.pop(cq)
            if cq == 0:
                # remaining weights/constants in first-use order; wv/wo land
                # during this quarter's Q-proj
                nc.sync.dma_start(wq_t[0][:], wq_d[:, 0:4096])
                nc.sync.dma_start(fmp[:, 0:512], fmp_d[:, 0:512])
                nc.sync.dma_start(wq_t[1][:], wq_d[:, 4096:8192])
                nc.sync.dma_start(wv_t[:], wv_d[:, :])
                nc.sync.dma_start(msk[:], msk_d[:, :])
                nc.sync.dma_start(idn[:], idn_d[:, :])
                nc.vector.memset(v_sb[:, :, 128:129], 1.0)
                wo_t = wp.tile([128, HPC, 2048], BF16, tag="wo")
                for i in range(2):
                    nc.sync.dma_start(
                        wo_t[:, i * 2 : (i + 1) * 2, :],
                        wo_d[:, i * 4096 : (i + 1) * 4096],
                    )
                nc.sync.dma_start(fmp[:, 512:S], fmp_d[:, 512:S])
            cs = slice(cq * 512, (cq + 1) * 512)
            fmc, fpc = fmp[0:64, cs], fmp[64:128, cs]
            blk = slice(cq * 4, (cq + 1) * 4)

            def q_rope(h, qps):
                hc = slice(h * 128, (h + 1) * 128)
                src = qps[0:64, :].rearrange("p (b q) -> p b q", b=4)
                nc.vector.tensor_mul(
                    q1[0:64, blk, hc], src, fmc.rearrange("p (b q) -> p b q", b=4)
                )
                nc.vector.tensor_mul(
                    q1[64:128, blk, hc], src, fpc.rearrange("p (b q) -> p b q", b=4)
                )
                nc.scalar.copy(
                    q2[0:64, blk, hc],
                    qps[64:128, :].rearrange("p (b q) -> p b q", b=4),
                )
                nc.scalar.copy(
                    q2[64:128, blk, hc],
                    qps[64:128, :].rearrange("p (b q) -> p b q", b=4),
                )

            # K projection + rope
            kps = ps.tile([128, 512], F32, tag="ps")
            for dc in range(NDC):
                nc.tensor.matmul(
                    kps[:],
                    wk_t[:, dc, :],
                    x_q[:, dc, :],
                    start=(dc == 0),
                    stop=(dc == NDC - 1),
                )
            nc.vector.tensor_mul(k1[0:64, cs], kps[0:64, :], fmc)
            nc.vector.tensor_mul(k1[64:128, cs], kps[0:64, :], fpc)
            nc.scalar.copy(k2[0:64, cs], kps[64:128, :])
            nc.scalar.copy(k2[64:128, cs], kps[64:128, :])

            # Q projections + rope, packed layout [feat, qblock, h*128+q]
            for h in range(HPC):
                qps = ps.tile([128, 512], F32, tag="ps")
                for dc in range(NDC):
                    nc.tensor.matmul(
                        qps[:],
                        wq_t[h // 2][:, dc, (h % 2) * 128 : (h % 2 + 1) * 128],
                        x_q[:, dc, :],
                        start=(dc == 0),
                        stop=(dc == NDC - 1),
                    )
                q_rope(h, qps)

            # V projection -> v_sb [kpos, feat] (first consumed mid-way into
            # this quarter's attention, so it sits after Q to shorten the
            # rope -> first-scores critical chain)
            vps = ps.tile([128, 512], F32, tag="ps")
            for kb4 in range(4):
                for dc in range(NDC):
                    nc.tensor.matmul(
                        vps[:, kb4 * 128 : (kb4 + 1) * 128],
                        x_q[:, dc, kb4 * 128 : (kb4 + 1) * 128],
                        wv_t[:, dc, :],
                        start=(dc == 0),
                        stop=(dc == NDC - 1),
                    )
            nc.vector.tensor_copy(
                v_sb[:, cq * 4 : (cq + 1) * 4, 0:128],
                vps[:].rearrange("p (b q) -> p b q", b=4),
            )

            # prefetch next x chunk during this quarter's attention
            if cq + 1 < 4:
                x_tiles[cq + 1] = load_x(cq + 1, [8, 8])

            # finish the previous quarter's last q-block now: its transposes
            # + O-projection give the PE work while the rope tail (vector/
            # scalar) of this quarter completes
            if pending is not None:
                emit_finish(pending)
                pending = None

            # ---- attention for q-blocks of this quarter ----
            # quarter 0 runs [3,2,1,0]: the ramp q-blocks (1-2 score pairs)
            # can't hide their exp->mask->PV chains, so they sit against the
            # quarter boundary where cq1's projections cover them
            qcs = [3, 2, 1, 0] if cq == 0 else range(cq * 4, (cq + 1) * 4)
            for qc in qcs:
                kbs = _kblocks(qc)
                nkb = len(kbs)
                pv = [
                    pvs.tile([128, 264], F32, tag="pv", name=f"pv{qc}_{i}")
                    for i in range(2)
                ]
                # Two heads accumulate in one bank: a start=True matmul would
                # clear the co-resident head's has_written bits mid-group, so
                # zero the bank and accumulate with start=False throughout
                # (add-where-set on zeros / overwrite-where-clear both work).
                for t in pv:
                    nc.vector.memset(t[:], 0.0)

                # masked k-blocks (diagonal, window tail) first: their extra
                # post-exp mask multiply sits on the exp->PV chain, so keep
                # them off the q-block tail where that chain is exposed
                kbs_proc = [kbs[-1]] + ([kbs[0]] if nkb > 1 else []) + kbs[1:-1]
                for mi, kb in enumerate(kbs_proc):
                    sp = sps.tile([128, 512], F32, tag="s")
                    lo = kb * 128
                    nc.tensor.matmul(
                        sp[:], k1[:, lo : lo + 128], q1[:, qc, :],
                        start=True, stop=False,
                    )
                    nc.tensor.matmul(
                        sp[:], k2[:, lo : lo + 128], q2[:, qc, :],
                        start=False, stop=True,
                    )
                    p_sb = pp.tile([128, 512], BF16, tag="p")
                    nc.scalar.activation(p_sb[:], sp[:], EXP)
                    # sliding-window mask: 0/1 multiply after the exp, on the
                    # otherwise-idle GpSimd engine (SBUF-only op)
                    masked = kb == qc or kb == qc - 8
                    if kb == qc:
                        nc.gpsimd.tensor_mul(p_sb[:], p_sb[:], msk[:, 512:1024])
                    if kb == qc - 8:
                        nc.gpsimd.tensor_mul(p_sb[:], p_sb[:], msk[:, 0:512])
                    # the PV group trails by 4 k-blocks, carried across
                    # q-block boundaries so score matmuls always cover the
                    # exp latency
                    window.append((pv, kb, p_sb, masked))
                    # the previous q-block's finish advances one stage per
                    # k-block: each transpose trails the vector recip/norm
                    # chain with a score pair of PE cover in between
                    if pending is not None and mi >= 1:
                        if finish_step(pending):
                            pending = None
                    while len(window) > 4:
                        fire_one()
                while pending is not None:
                    if finish_step(pending):
                        pending = None
                pending = {
                    "pv": pv, "qc": qc, "at": None, "rc": None, "a": {},
                    "step": 0,
                }

        emit_finish(pending)

    nc.compile()
    return nc


def _prep_core(inputs, c):
    x = inputs["x"]
    cos, sin = np.asarray(inputs["cos"]), np.asarray(inputs["sin"])
    mask = np.asarray(inputs["mask"])
    wq = np.asarray(inputs["wq"], dtype=np.float32)
    wk = np.asarray(inputs["wk"], dtype=np.float32)
    wv = np.asarray(inputs["wv"], dtype=np.float32)
    wo = np.asarray(inputs["wo"], dtype=np.float32)
    bf = ml_dtypes.bfloat16
    b, g = c // 4, c % 4

    # x[b] transposed -> [128p, cq, dc, 512]
    xt = np.asarray(x[b], dtype=np.float32).T  # [dim, S]
    xt = xt.reshape(NDC, 128, 4, 512).transpose(1, 2, 0, 3)
    xt = np.ascontiguousarray(xt).reshape(128, 4 * NDC * 512).astype(bf)

    # wq slice for heads 4g..4g+3 (SCALE folded). The imag half-columns are
    # additionally halved: the kernel duplicates q2/k2 across both partition
    # halves, doubling the imag contraction.
    wqs = (wq[:, g * 512 : (g + 1) * 512] * SCALE).reshape(DIM, HPC, 128).copy()
    wqs[:, :, 64:128] *= 0.5
    wqs = wqs.reshape(NDC, 128, 2, 256)
    wqs = np.ascontiguousarray(wqs.transpose(1, 2, 0, 3)).reshape(128, 2 * NDC * 256)
    # wk / wv slices for kv head g: [p, dc, 128]
    wks = wk[:, g * 128 : (g + 1) * 128].reshape(NDC, 128, 128).transpose(1, 0, 2)
    wks = np.ascontiguousarray(wks).reshape(128, NDC * 128)
    wvs = wv[:, g * 128 : (g + 1) * 128].reshape(NDC, 128, 128).transpose(1, 0, 2)
    wvs = np.ascontiguousarray(wvs).reshape(128, NDC * 128)
    # wo rows for this core's heads: [p, h, 2048] tiles
    wos = wo[g * 512 : (g + 1) * 512].reshape(HPC, 128, 2048).transpose(1, 0, 2)
    wos = np.ascontiguousarray(wos).reshape(128, HPC * 2048)

    # fm rows 0:64, fp rows 64:128 (one tensor -> one DMA)
    fmp = np.concatenate([(cos - sin).T, (cos + sin).T], axis=0)
    fmp = np.ascontiguousarray(fmp, dtype=np.float32).astype(bf)
    # 0/1 keep-masks, transposed for the S^T layout, tiled across the 4
    # packed heads: [tail block | diagonal block]
    tail01 = (mask[WINDOW : WINDOW + 128, 0:128].T == 0.0).astype(np.float32)
    diag01 = (mask[0:128, 0:128].T == 0.0).astype(np.float32)
    msk = np.concatenate([np.tile(tail01, (1, 4)), np.tile(diag01, (1, 4))], axis=1)
    msk = np.ascontiguousarray(msk).astype(bf)
    idn = np.ascontiguousarray(np.eye(128, dtype=np.float32)).astype(bf)

    return {
        "xt": xt, "wq": wqs.astype(bf), "wk": wks.astype(bf), "wv": wvs.astype(bf),
        "wo": wos.astype(bf), "fmp": fmp, "msk": msk, "idn": idn,
    }


def kernel(**inputs) -> np.ndarray:
    if "nc" not in _cache:
        _cache["nc"] = _build()
    nc = _cache["nc"]
    in_maps = [_prep_core(inputs, c) for c in range(8)]
    res = run_bass_kernel_spmd(nc, in_maps, core_ids=list(range(8)))
    out = np.zeros((B, S, DIM), dtype=np.float32)
    for c in range(8):
        out[c // 4] += np.asarray(res.results[c]["out"], dtype=np.float32)
    return out


# revision 49
# speedup vs baseline: 1.0045x; 1.0008x over previous
"""Distributed Bass kernel for sliding-window GQA attention on 8 TRN2 NeuronCores.

Problem: B=2, S=2048, DIM=2048, H=16, KVH=4, HD=128, WINDOW=1024 (causal
sliding window), nonstandard RoPE producing 1.5*HD score features.

Sharding (tensor-parallel on the kv-head axis, data-parallel on batch —
no collectives): core c owns (batch, kv-group) = (c//4, c%4): its 4 q-heads
and 1 kv head over the full 2048-row sequence. wq/wk/wv are column-sharded
by kv group, wo row-sharded. Each core emits a PARTIAL output projection
(its 4 heads x its wo rows); the host sums the 4 partials per batch while
unsharding — replacing the all-reduce.

Structure: scores are computed TRANSPOSED (S^T[k, q], k on partitions) with
all 4 heads packed into one N=512 matmul pair per k-block — q1/k1 are
feature-major already so this is free. The imag-half (64-dim) contraction
is duplicated across both partition halves (wq imag columns pre-halved on
the host) so both score passes run K=128: a 64-row stationary gets a
row_grp-masked LDWEIGHTS that cannot overlap the in-flight matmul
(measured +210ns per pair). Sliding-window masking is a post-exp 0/1 bf16
multiply on the otherwise-idle GpSimd engine (replacing pre-exp -1e9 adds
on the congested Vector engine); masked PV groups are fired LAST from the
trailing window so the ~1.2us GpSimd latency is always covered. The
softmax row-sums come from a ones-column appended to V (PV out [q, 129]
carries the denominator in col 128), normalization happens during the
PSUM->SBUF attn copy (per-partition scalar mul), and a single 128x128
transpose matmul per (head, q-block) feeds the O-projection.

Pipelining: attention for q-blocks 4cq..4cq+3 is interleaved right after
column-chunk cq's projections; PV groups trail the score matmuls by 4
k-blocks and carry across q-block boundaries (so the exp on the Scalar
engine never stalls the in-order PE queue). The staged finish of q-block
qc advances one step per k-block inside qc+1's loop: norms (vector) lead
their transposes (PE) by a full step, and the transpose scratch comes from
the ps pool, not sps, so the finish chain never back-pressures the score
banks. Quarter 0 runs its q-blocks [3,2,1,0] so the 1-2-pair ramp blocks
sit against the quarter boundary where cq1's projections hide their
exp->mask->PV chains.

Prologue: the PE warm-up (HAM clock-gate lift) and exp-table prewarm run on
a memset tile with no DMA dependency, so they start as soon as the engines
initialize (~6.5us) instead of after the first weight DMA lands (~12us).
DMA issue order is strictly by first-use with few mid-sized issues (each
dma_start costs 0.6-1.2us of serial Sync time; concurrent transfers share
~400GB/s equally per outstanding issue): wk, x0 in 3 parts, wq halves,
fmp, wv, masks/identity, wo. Outputs go out as two fused bf16 DMAs per
q-block (four for the final q-block so the store overlaps the last
O-proj).
"""
import numpy as np
import ml_dtypes

import concourse.tile as tile
from concourse import bacc, mybir
from concourse.bass_utils import run_bass_kernel_spmd
from contextlib import ExitStack

F32 = mybir.dt.float32
BF16 = mybir.dt.bfloat16
EXP = mybir.ActivationFunctionType.Exp

B, S, DIM = 2, 2048, 2048
H, KVH, HD = 16, 4, 128
HPC = H // KVH  # heads per core (4)
WINDOW = 1024
SCALE = HD ** -0.5
NDC = DIM // 128  # 16 dim chunks
NQC = S // 128    # 16 q blocks

_cache = {}


def _kblocks(qc):
    return list(range(max(0, qc - 8), qc + 1))


def _build():
    nc = bacc.Bacc("TRN2", target_bir_lowering=False, debug=False, num_devices=8)

    xt_d = nc.dram_tensor("xt", [128, 4 * NDC * 512], BF16, kind="ExternalInput")
    wq_d = nc.dram_tensor("wq", [128, 2 * NDC * 256], BF16, kind="ExternalInput")
    wk_d = nc.dram_tensor("wk", [128, NDC * 128], BF16, kind="ExternalInput")
    wv_d = nc.dram_tensor("wv", [128, NDC * 128], BF16, kind="ExternalInput")
    wo_d = nc.dram_tensor("wo", [128, HPC * 2048], BF16, kind="ExternalInput")
    fmp_d = nc.dram_tensor("fmp", [128, S], BF16, kind="ExternalInput")
    msk_d = nc.dram_tensor("msk", [128, 1024], BF16, kind="ExternalInput")
    idn_d = nc.dram_tensor("idn", [128, 128], BF16, kind="ExternalInput")
    out_d = nc.dram_tensor("out", [S, DIM], BF16, kind="ExternalOutput")

    with tile.TileContext(nc) as tc, ExitStack() as ctx:
        xp = ctx.enter_context(tc.tile_pool(name="xp", bufs=3))
        wp = ctx.enter_context(tc.tile_pool(name="wp", bufs=1))
        cp = ctx.enter_context(tc.tile_pool(name="cp", bufs=1))
        qp = ctx.enter_context(tc.tile_pool(name="qp", bufs=1))
        kp = ctx.enter_context(tc.tile_pool(name="kp", bufs=1))
        vp = ctx.enter_context(tc.tile_pool(name="vp", bufs=1))
        pp = ctx.enter_context(tc.tile_pool(name="pp", bufs=12))
        ap_ = ctx.enter_context(tc.tile_pool(name="ap", bufs=8))
        atp = ctx.enter_context(tc.tile_pool(name="atp", bufs=2))
        rp = ctx.enter_context(tc.tile_pool(name="rp", bufs=8))
        op_ = ctx.enter_context(tc.tile_pool(name="op", bufs=2))
        # PSUM: 8 banks = ps(2: proj + O-proj) + sps(3: scores + attn
        # transposes) + pvs(3: PV accumulators, 2 heads per bank)
        ps = ctx.enter_context(tc.tile_pool(name="ps", bufs=2, space="PSUM"))
        sps = ctx.enter_context(tc.tile_pool(name="sps", bufs=3, space="PSUM"))
        pvs = ctx.enter_context(tc.tile_pool(name="pvs", bufs=3, space="PSUM"))

        # ---- persistent SBUF tensors ----
        # k2/q2 duplicate the imag half across both partition halves (wq imag
        # columns pre-halved on the host): a 64-row stationary would get a
        # row_grp-masked LDWEIGHTS that cannot overlap the in-flight matmul
        # (measured +210ns per score pair), so both passes stay K=128.
        q1 = qp.tile([128, NQC, 512], BF16, tag="q1")  # [feat, qblock, h*128+q]
        q2 = qp.tile([128, NQC, 512], BF16, tag="q2")  # imag, duplicated halves
        k1 = kp.tile([128, S], BF16, tag="k1")
        k2 = kp.tile([128, S], BF16, tag="k2")  # imag, duplicated halves
        v_sb = vp.tile([128, NQC, 132], BF16, tag="v")  # col 128 = ones

        # warm-up source with no DMA dependency: PE warm-up matmuls lift the
        # HAM clock gate and the first activation pulls the exp table-set
        # (~2.7us) while the first input DMAs are still in flight
        wz = cp.tile([128, 512], BF16, tag="wz")
        nc.vector.memset(wz[:], 0.0)
        warm = rp.tile([128, 1], F32, tag="rc", name="warm")
        nc.scalar.activation(warm[:], wz[:, 0:1], EXP)
        # 12 N=512 matmuls = ~5.1us of sustained PE activity at the cold
        # rate: covers 1.5 free-running HAM SHORT windows so the clock gate
        # reliably lifts to 2.4 GHz before the Q-projection (an 8-matmul
        # burst spans just one window and missed the flip on most runs,
        # leaving projections at half rate until ~20us). Costs nothing: the
        # K-projection is DMA-paced until past this point anyway.
        wups = ps.tile([128, 512], F32, tag="ps", name="wups")
        for i in range(12):
            nc.tensor.matmul(
                wups[:], wz[:, 0:128], wz[:], start=(i == 0), stop=(i == 11),
            )

        # ---- DMA issue order = transfer order: strictly by first use ----
        wk_t = wp.tile([128, NDC, 128], BF16, tag="wk")
        wv_t = wp.tile([128, NDC, 128], BF16, tag="wv")

        def load_x(cq, parts):
            x_q = xp.tile([128, NDC, 512], BF16, tag="x", name=f"x{cq}")
            dg = 0
            for w_dg in parts:
                nc.sync.dma_start(
                    x_q[:, dg : dg + w_dg, :],
                    xt_d[
                        :,
                        cq * NDC * 512 + dg * 512 : cq * NDC * 512
                        + (dg + w_dg) * 512,
                    ],
                )
                dg += w_dg
            return x_q

        # front DMA order = first-use order. Few, mid-sized issues: each
        # dma_start costs 0.6-1.2us of serial Sync time, and concurrent
        # transfers share bandwidth round-robin, so a long tail of small
        # parts delays everything behind it (measured x0 landing at 20us)
        # transfers share bandwidth equally per outstanding dma_start, so the
        # split also acts as a priority: x0 keeps 2-3 queues against wq's one
        nc.sync.dma_start(wk_t[:], wk_d[:, :])
        x_q0 = xp.tile([128, NDC, 512], BF16, tag="x", name="x0")
        wq_t = [
            wp.tile([128, NDC, 256], BF16, tag=f"wq{i}", name=f"wq{i}")
            for i in range(2)
        ]
        nc.sync.dma_start(x_q0[:, 0:2, :], xt_d[:, 0:1024])
        nc.sync.dma_start(x_q0[:, 2:8, :], xt_d[:, 1024:4096])
        nc.sync.dma_start(x_q0[:, 8:16, :], xt_d[:, 4096:8192])
        x_tiles = {0: x_q0}

        fmp = cp.tile([128, S], BF16, tag="fmp")  # fm rows 0:64, fp 64:128
        msk = cp.tile([128, 1024], BF16, tag="msk")  # [tail 0:512 | diag 512:1024]
        idn = cp.tile([128, 128], BF16, tag="idn")
        wo_t = None
        pending = None  # (pv tiles, qc) awaiting normalize/transpose/O-proj
        window = []  # trailing (pv, kb, p_sb, masked) PV groups, oldest first

        def pv_group(pv, kb, p_sb, is_stop):
            for h in range(HPC):
                nc.tensor.matmul(
                    pv[h // 2][:, (h % 2) * 132 : (h % 2) * 132 + 129],
                    p_sb[:, h * 128 : (h + 1) * 128],
                    v_sb[:, kb, 0:129],
                    start=False,
                    stop=is_stop,
                )

        def fire(idx):
            pv, kb, p_sb, masked = window.pop(idx)
            is_stop = not any(w[0] is pv for w in window)
            pv_group(pv, kb, p_sb, is_stop)

        def fire_one():
            # prefer the oldest UNMASKED group: masked p tiles wait on the
            # GpSimd mask multiply (~1.2us), so they fire as late as
            # possible — accumulation order into the PV bank is free
            for j, w in enumerate(window):
                if not w[3]:
                    fire(j)
                    return
            fire(0)

        def flush_to(pv_tiles):
            # a finish reads its PV accumulators: every trailing group that
            # targets them must be emitted first (masked ones last)
            while True:
                idxs = [j for j, w in enumerate(window) if w[0] is pv_tiles]
                if not idxs:
                    return
                unm = [j for j in idxs if not window[j][3]]
                fire(unm[0] if unm else idxs[0])

        def finish_norm(st, h):
            # head h's normalize on the vector engine
            pv = st["pv"]
            c0 = (h % 2) * 132
            if h % 2 == 0:
                # both heads' softmax denominators sit at cols 128/260 of the
                # shared bank: one strided reciprocal covers the pair
                rc = rp.tile([128, 2], F32, tag="rc")
                st["rc"] = rc
                nc.vector.reciprocal(rc[:], pv[h // 2][:, 128:261:132])
            rc = st["rc"]
            a_sb = ap_.tile([128, 128], BF16, tag="a")
            st["a"][h] = a_sb
            nc.vector.tensor_scalar_mul(
                a_sb[:], pv[h // 2][:, c0 : c0 + 128], rc[:, h % 2 : h % 2 + 1]
            )

        def finish_trans(st, h):
            # head h's transpose, one step AFTER its norm: the PE-side
            # transpose otherwise waits in-step on the vector norm chain
            # (measured ~670ns stall per head)
            if st["at"] is None:
                st["at"] = atp.tile([128, HPC, 128], BF16, tag="at", name="at_sb")
            at_sb = st["at"]
            # transpose scratch comes from the ps pool (idle until the O-proj
            # step), NOT sps: an sps tile would couple the finish chain into
            # the score-pair banks via WAR deps
            tps = ps.tile([128, 512], F32, tag="ps", name=f"tp{h}")
            nc.tensor.matmul(tps[:, 0:128], st["a"][h], idn, start=True, stop=True)
            # all 4 copies on vector: scalar is the busier engine during
            # attention (one 690ns exp per pair) and queued copies there
            # delay exps, which shows up as score-matmul stalls
            nc.vector.tensor_copy(at_sb[:, h, :], tps[:, 0:128])

        def finish_tail(st):
            at_sb = st["at"]
            qb = st["qc"] * 128
            last = st["qc"] == NQC - 1
            o_sb = op_.tile([128, 2048], BF16, tag="o")
            for dn in range(4):
                ops = ps.tile([128, 512], F32, tag="ps")
                for f in range(HPC):
                    nc.tensor.matmul(
                        ops[:],
                        at_sb[:, f, :],
                        wo_t[:, f, dn * 512 : (dn + 1) * 512],
                        start=(f == 0),
                        stop=(f == HPC - 1),
                    )
                nc.vector.tensor_copy(o_sb[:, dn * 512 : (dn + 1) * 512], ops[:])
                if last:
                    # final q-block: store each quarter as soon as its copy
                    # lands so the DMA overlaps the remaining O-proj matmuls
                    nc.sync.dma_start(
                        out_d[qb : qb + 128, dn * 512 : (dn + 1) * 512],
                        o_sb[:, dn * 512 : (dn + 1) * 512],
                    )
            # one fused DMA per q-block: halves the serial Sync issue cost;
            # the later store start is harmless (op_ pool has ~10us of slack
            # and output bandwidth is plentiful mid-kernel)
            if not last:
                nc.sync.dma_start(out_d[qb : qb + 128, :], o_sb[:])

        def finish_step(st):
            # advance the staged finish by one unit; True when fully done.
            # Stages: 0: flush+norm0, 1-3: norm h / transpose h-1, 4:
            # transpose 3, 5: O-proj tail
            s = st["step"]
            if s == 0:
                flush_to(st["pv"])
                finish_norm(st, 0)
            elif s <= 3:
                finish_norm(st, s)
                finish_trans(st, s - 1)
            elif s == 4:
                finish_trans(st, 3)
            else:
                finish_tail(st)
            st["step"] += 1
            return st["step"] > HPC + 1

        def emit_finish(st):
            while not finish_step(st):
                pass

        for cq in range(4):
            # ---- projections for column-quarter cq ----
            x_q = x_tiles.pop(cq)
            if cq == 0:
                # remaining weights/constants in first-use order; wv/wo land
                # during this quarter's Q-proj
                nc.sync.dma_start(wq_t[0][:], wq_d[:, 0:4096])
                nc.sync.dma_start(fmp[:, 0:512], fmp_d[:, 0:512])
                nc.sync.dma_start(wq_t[1][:], wq_d[:, 4096:8192])
                nc.sync.dma_start(wv_t[:], wv_d[:, :])
                nc.sync.dma_start(msk[:], msk_d[:, :])
                nc.sync.dma_start(idn[:], idn_d[:, :])
                nc.vector.memset(v_sb[:, :, 128:129], 1.0)
                wo_t = wp.tile([128, HPC, 2048], BF16, tag="wo")
                for i in range(2):
                    nc.sync.dma_start(
                        wo_t[:, i * 2 : (i + 1) * 2, :],
                        wo_d[:, i * 4096 : (i + 1) * 4096],
                    )
                nc.sync.dma_start(fmp[:, 512:S], fmp_d[:, 512:S])
            cs = slice(cq * 512, (cq + 1) * 512)
            fmc, fpc = fmp[0:64, cs], fmp[64:128, cs]
            blk = slice(cq * 4, (cq + 1) * 4)

            def q_rope(h, qps):
                hc = slice(h * 128, (h + 1) * 128)
                src = qps[0:64, :].rearrange("p (b q) -> p b q", b=4)
                nc.vector.tensor_mul(
                    q1[0:64, blk, hc], src, fmc.rearrange("p (b q) -> p b q", b=4)
                )
                nc.vector.tensor_mul(
                    q1[64:128, blk, hc], src, fpc.rearrange("p (b q) -> p b q", b=4)
                )
                nc.scalar.copy(
                    q2[0:64, blk, hc],
                    qps[64:128, :].rearrange("p (b q) -> p b q", b=4),
                )
                nc.scalar.copy(
                    q2[64:128, blk, hc],
                    qps[64:128, :].rearrange("p (b q) -> p b q", b=4),
                )

            # K projection + rope
            kps = ps.tile([128, 512], F32, tag="ps")
            for dc in range(NDC):
                nc.tensor.matmul(
                    kps[:],
                    wk_t[:, dc, :],
                    x_q[:, dc, :],
                    start=(dc == 0),
                    stop=(dc == NDC - 1),
                )
            nc.vector.tensor_mul(k1[0:64, cs], kps[0:64, :], fmc)
            nc.vector.tensor_mul(k1[64:128, cs], kps[0:64, :], fpc)
            nc.scalar.copy(k2[0:64, cs], kps[64:128, :])
            nc.scalar.copy(k2[64:128, cs], kps[64:128, :])

            # Q projections + rope, packed layout [feat, qblock, h*128+q]
            for h in range(HPC):
                qps = ps.tile([128, 512], F32, tag="ps")
                for dc in range(NDC):
                    nc.tensor.matmul(
                        qps[:],
                        wq_t[h // 2][:, dc, (h % 2) * 128 : (h % 2 + 1) * 128],
                        x_q[:, dc, :],
                        start=(dc == 0),
                        stop=(dc == NDC - 1),
                    )
                q_rope(h, qps)

            # V projection -> v_sb [kpos, feat] (first consumed mid-way into
            # this quarter's attention, so it sits after Q to shorten the
            # rope -> first-scores critical chain)
            vps = ps.tile([128, 512], F32, tag="ps")
            for kb4 in range(4):
                for dc in range(NDC):
                    nc.tensor.matmul(
                        vps[:, kb4 * 128 : (kb4 + 1) * 128],
                        x_q[:, dc, kb4 * 128 : (kb4 + 1) * 128],
                        wv_t[:, dc, :],
                        start=(dc == 0),
                        stop=(dc == NDC - 1),
                    )
            nc.vector.tensor_copy(
                v_sb[:, cq * 4 : (cq + 1) * 4, 0:128],
                vps[:].rearrange("p (b q) -> p b q", b=4),
            )

            # prefetch next x chunk during this quarter's attention (one
            # issue: it saves serial Sync time and the consumer is a full
            # quarter away)
            if cq + 1 < 4:
                x_tiles[cq + 1] = load_x(cq + 1, [16])

            # finish the previous quarter's last q-block now: its transposes
            # + O-projection give the PE work while the rope tail (vector/
            # scalar) of this quarter completes
            if pending is not None:
                emit_finish(pending)
                pending = None

            # ---- attention for q-blocks of this quarter ----
            # quarter 0 runs [3,2,1,0]: the ramp q-blocks (1-2 score pairs)
            # can't hide their exp->mask->PV chains, so they sit against the
            # quarter boundary where cq1's projections cover them
            qcs = [3, 2, 1, 0] if cq == 0 else range(cq * 4, (cq + 1) * 4)
            for qc in qcs:
                kbs = _kblocks(qc)
                nkb = len(kbs)
                pv = [
                    pvs.tile([128, 264], F32, tag="pv", name=f"pv{qc}_{i}")
                    for i in range(2)
                ]
                # Two heads accumulate in one bank: a start=True matmul would
                # clear the co-resident head's has_written bits mid-group, so
                # zero the bank and accumulate with start=False throughout
                # (add-where-set on zeros / overwrite-where-clear both work).
                for t in pv:
                    nc.vector.memset(t[:], 0.0)

                # masked k-blocks (diagonal, window tail) first: their extra
                # post-exp mask multiply sits on the exp->PV chain, so keep
                # them off the q-block tail where that chain is exposed
                kbs_proc = [kbs[-1]] + ([kbs[0]] if nkb > 1 else []) + kbs[1:-1]
                for mi, kb in enumerate(kbs_proc):
                    sp = sps.tile([128, 512], F32, tag="s")
                    lo = kb * 128
                    nc.tensor.matmul(
                        sp[:], k1[:, lo : lo + 128], q1[:, qc, :],
                        start=True, stop=False,
                    )
                    nc.tensor.matmul(
                        sp[:], k2[:, lo : lo + 128], q2[:, qc, :],
                        start=False, stop=True,
                    )
                    p_sb = pp.tile([128, 512], BF16, tag="p")
                    nc.scalar.activation(p_sb[:], sp[:], EXP)
                    # sliding-window mask: 0/1 multiply after the exp, on the
                    # otherwise-idle GpSimd engine (SBUF-only op)
                    masked = kb == qc or kb == qc - 8
                    if kb == qc:
                        nc.gpsimd.tensor_mul(p_sb[:], p_sb[:], msk[:, 512:1024])
                    if kb == qc - 8:
                        nc.gpsimd.tensor_mul(p_sb[:], p_sb[:], msk[:, 0:512])
                    # the PV group trails by 4 k-blocks, carried across
                    # q-block boundaries so score matmuls always cover the
                    # exp latency
                    window.append((pv, kb, p_sb, masked))
                    # the previous q-block's finish advances one stage per
                    # k-block: each transpose trails the vector recip/norm
                    # chain with a score pair of PE cover in between
                    if pending is not None and mi >= 1:
                        if finish_step(pending):
                            pending = None
                    while len(window) > 4:
                        fire_one()
                while pending is not None:
                    if finish_step(pending):
                        pending = None
                pending = {
                    "pv": pv, "qc": qc, "at": None, "rc": None, "a": {},
                    "step": 0,
                }

        emit_finish(pending)

    nc.compile()
    return nc


def _prep_core(inputs, c):
    x = inputs["x"]
    cos, sin = np.asarray(inputs["cos"]), np.asarray(inputs["sin"])
    mask = np.asarray(inputs["mask"])
    wq = np.asarray(inputs["wq"], dtype=np.float32)
    wk = np.asarray(inputs["wk"], dtype=np.float32)
    wv = np.asarray(inputs["wv"], dtype=np.float32)
    wo = np.asarray(inputs["wo"], dtype=np.float32)
    bf = ml_dtypes.bfloat16
    b, g = c // 4, c % 4

    # x[b] transposed -> [128p, cq, dc, 512]
    xt = np.asarray(x[b], dtype=np.float32).T  # [dim, S]
    xt = xt.reshape(NDC, 128, 4, 512).transpose(1, 2, 0, 3)
    xt = np.ascontiguousarray(xt).reshape(128, 4 * NDC * 512).astype(bf)

    # wq slice for heads 4g..4g+3 (SCALE folded). The imag half-columns are
    # additionally halved: the kernel duplicates q2/k2 across both partition
    # halves, doubling the imag contraction.
    wqs = (wq[:, g * 512 : (g + 1) * 512] * SCALE).reshape(DIM, HPC, 128).copy()
    wqs[:, :, 64:128] *= 0.5
    wqs = wqs.reshape(NDC, 128, 2, 256)
    wqs = np.ascontiguousarray(wqs.transpose(1, 2, 0, 3)).reshape(128, 2 * NDC * 256)
    # wk / wv slices for kv head g: [p, dc, 128]
    wks = wk[:, g * 128 : (g + 1) * 128].reshape(NDC, 128, 128).transpose(1, 0, 2)
    wks = np.ascontiguousarray(wks).reshape(128, NDC * 128)
    wvs = wv[:, g * 128 : (g + 1) * 128].reshape(NDC, 128, 128).transpose(1, 0, 2)
    wvs = np.ascontiguousarray(wvs).reshape(128, NDC * 128)
    # wo rows for this core's heads: [p, h, 2048] tiles
    wos = wo[g * 512 : (g + 1) * 512].reshape(HPC, 128, 2048).transpose(1, 0, 2)
    wos = np.ascontiguousarray(wos).reshape(128, HPC * 2048)

    # fm rows 0:64, fp rows 64:128 (one tensor -> one DMA)
    fmp = np.concatenate([(cos - sin).T, (cos + sin).T], axis=0)
    fmp = np.ascontiguousarray(fmp, dtype=np.float32).astype(bf)
    # 0/1 keep-masks, transposed for the S^T layout, tiled across the 4
    # packed heads: [tail block | diagonal block]
    tail01 = (mask[WINDOW : WINDOW + 128, 0:128].T == 0.0).astype(np.float32)
    diag01 = (mask[0:128, 0:128].T == 0.0).astype(np.float32)
    msk = np.concatenate([np.tile(tail01, (1, 4)), np.tile(diag01, (1, 4))], axis=1)
    msk = np.ascontiguousarray(msk).astype(bf)
    idn = np.ascontiguousarray(np.eye(128, dtype=np.float32)).astype(bf)

    return {
        "xt": xt, "wq": wqs.astype(bf), "wk": wks.astype(bf), "wv": wvs.astype(bf),
        "wo": wos.astype(bf), "fmp": fmp, "msk": msk, "idn": idn,
    }


def kernel(**inputs) -> np.ndarray:
    if "nc" not in _cache:
        _cache["nc"] = _build()
    nc = _cache["nc"]
    in_maps = [_prep_core(inputs, c) for c in range(8)]
    res = run_bass_kernel_spmd(nc, in_maps, core_ids=list(range(8)))
    out = np.zeros((B, S, DIM), dtype=np.float32)
    for c in range(8):
        out[c // 4] += np.asarray(res.results[c]["out"], dtype=np.float32)
    return out


# revision 51
# speedup vs baseline: 1.0111x; 1.0066x over previous
"""Distributed Bass kernel for sliding-window GQA attention on 8 TRN2 NeuronCores.

Problem: B=2, S=2048, DIM=2048, H=16, KVH=4, HD=128, WINDOW=1024 (causal
sliding window), nonstandard RoPE producing 1.5*HD score features.

Sharding (tensor-parallel on the kv-head axis, data-parallel on batch —
no collectives): core c owns (batch, kv-group) = (c//4, c%4): its 4 q-heads
and 1 kv head over the full 2048-row sequence. wq/wk/wv are column-sharded
by kv group, wo row-sharded. Each core emits a PARTIAL output projection
(its 4 heads x its wo rows); the host sums the 4 partials per batch while
unsharding — replacing the all-reduce.

Structure: scores are computed TRANSPOSED (S^T[k, q], k on partitions) with
all 4 heads packed into one N=512 matmul pair per k-block — q1/k1 are
feature-major already so this is free. The imag-half (64-dim) contraction
is duplicated across both partition halves (wq imag columns pre-halved on
the host) so both score passes run K=128: a 64-row stationary gets a
row_grp-masked LDWEIGHTS that cannot overlap the in-flight matmul
(measured +210ns per pair). Sliding-window masking is a post-exp 0/1 bf16
multiply on the otherwise-idle GpSimd engine (replacing pre-exp -1e9 adds
on the congested Vector engine); masked PV groups are fired LAST from the
trailing window so the ~1.2us GpSimd latency is always covered. The
softmax row-sums come from a ones-column appended to V (PV out [q, 129]
carries the denominator in col 128), normalization happens during the
PSUM->SBUF attn copy (per-partition scalar mul), and a single 128x128
transpose matmul per (head, q-block) feeds the O-projection.

Pipelining: attention for q-blocks 4cq..4cq+3 is interleaved right after
column-chunk cq's projections; PV groups trail the score matmuls by 4
k-blocks and carry across q-block boundaries (so the exp on the Scalar
engine never stalls the in-order PE queue). The staged finish of q-block
qc advances one step per k-block inside qc+1's loop: norms (vector) lead
their transposes (PE) by a full step, and the transpose scratch comes from
the ps pool, not sps, so the finish chain never back-pressures the score
banks. Quarter 0 runs its q-blocks [3,2,1,0] so the 1-2-pair ramp blocks
sit against the quarter boundary where cq1's projections hide their
exp->mask->PV chains.

Prologue: the PE warm-up (HAM clock-gate lift) and exp-table prewarm run on
a memset tile with no DMA dependency, so they start as soon as the engines
initialize (~6.5us) instead of after the first weight DMA lands (~12us).
DMA issue order is strictly by first-use with few mid-sized issues (each
dma_start costs 0.6-1.2us of serial Sync time; concurrent transfers share
~400GB/s equally per outstanding issue): wk, x0 in 3 parts, wq halves,
fmp, wv, masks/identity, wo. Outputs go out as two fused bf16 DMAs per
q-block (four for the final q-block so the store overlaps the last
O-proj).
"""
import numpy as np
import ml_dtypes

import concourse.tile as tile
from concourse import bacc, mybir
from concourse.bass_utils import run_bass_kernel_spmd
from contextlib import ExitStack

F32 = mybir.dt.float32
BF16 = mybir.dt.bfloat16
EXP = mybir.ActivationFunctionType.Exp

B, S, DIM = 2, 2048, 2048
H, KVH, HD = 16, 4, 128
HPC = H // KVH  # heads per core (4)
WINDOW = 1024
SCALE = HD ** -0.5
NDC = DIM // 128  # 16 dim chunks
NQC = S // 128    # 16 q blocks

_cache = {}


def _kblocks(qc):
    return list(range(max(0, qc - 8), qc + 1))


def _build():
    nc = bacc.Bacc("TRN2", target_bir_lowering=False, debug=False, num_devices=8)

    xt_d = nc.dram_tensor("xt", [128, 4 * NDC * 512], BF16, kind="ExternalInput")
    wq_d = nc.dram_tensor("wq", [128, 2 * NDC * 256], BF16, kind="ExternalInput")
    wk_d = nc.dram_tensor("wk", [128, NDC * 128], BF16, kind="ExternalInput")
    wv_d = nc.dram_tensor("wv", [128, NDC * 128], BF16, kind="ExternalInput")
    wo_d = nc.dram_tensor("wo", [128, HPC * 2048], BF16, kind="ExternalInput")
    fmp_d = nc.dram_tensor("fmp", [128, S], BF16, kind="ExternalInput")
    msk_d = nc.dram_tensor("msk", [128, 1024], BF16, kind="ExternalInput")
    idn_d = nc.dram_tensor("idn", [128, 128], BF16, kind="ExternalInput")
    out_d = nc.dram_tensor("out", [S, DIM], BF16, kind="ExternalOutput")

    with tile.TileContext(nc) as tc, ExitStack() as ctx:
        xp = ctx.enter_context(tc.tile_pool(name="xp", bufs=3))
        wp = ctx.enter_context(tc.tile_pool(name="wp", bufs=1))
        cp = ctx.enter_context(tc.tile_pool(name="cp", bufs=1))
        qp = ctx.enter_context(tc.tile_pool(name="qp", bufs=1))
        kp = ctx.enter_context(tc.tile_pool(name="kp", bufs=1))
        vp = ctx.enter_context(tc.tile_pool(name="vp", bufs=1))
        pp = ctx.enter_context(tc.tile_pool(name="pp", bufs=12))
        ap_ = ctx.enter_context(tc.tile_pool(name="ap", bufs=8))
        atp = ctx.enter_context(tc.tile_pool(name="atp", bufs=2))
        rp = ctx.enter_context(tc.tile_pool(name="rp", bufs=8))
        op_ = ctx.enter_context(tc.tile_pool(name="op", bufs=2))
        # PSUM: 8 banks = ps(2: proj + O-proj) + sps(3: scores + attn
        # transposes) + pvs(3: PV accumulators, 2 heads per bank)
        ps = ctx.enter_context(tc.tile_pool(name="ps", bufs=2, space="PSUM"))
        sps = ctx.enter_context(tc.tile_pool(name="sps", bufs=3, space="PSUM"))
        pvs = ctx.enter_context(tc.tile_pool(name="pvs", bufs=3, space="PSUM"))

        # ---- persistent SBUF tensors ----
        # k2/q2 duplicate the imag half across both partition halves (wq imag
        # columns pre-halved on the host): a 64-row stationary would get a
        # row_grp-masked LDWEIGHTS that cannot overlap the in-flight matmul
        # (measured +210ns per score pair), so both passes stay K=128.
        q1 = qp.tile([128, NQC, 512], BF16, tag="q1")  # [feat, qblock, h*128+q]
        q2 = qp.tile([128, NQC, 512], BF16, tag="q2")  # imag, duplicated halves
        k1 = kp.tile([128, S], BF16, tag="k1")
        k2 = kp.tile([128, S], BF16, tag="k2")  # imag, duplicated halves
        v_sb = vp.tile([128, NQC, 132], BF16, tag="v")  # col 128 = ones

        # warm-up source with no DMA dependency: PE warm-up matmuls lift the
        # HAM clock gate and the first activation pulls the exp table-set
        # (~2.7us) while the first input DMAs are still in flight
        wz = cp.tile([128, 512], BF16, tag="wz")
        nc.vector.memset(wz[:], 0.0)
        warm = rp.tile([128, 1], F32, tag="rc", name="warm")
        nc.scalar.activation(warm[:], wz[:, 0:1], EXP)
        # 12 N=512 matmuls = ~5.1us of sustained PE activity at the cold
        # rate: covers 1.5 free-running HAM SHORT windows so the clock gate
        # reliably lifts to 2.4 GHz before the Q-projection (an 8-matmul
        # burst spans just one window and missed the flip on most runs,
        # leaving projections at half rate until ~20us). Costs nothing: the
        # K-projection is DMA-paced until past this point anyway.
        wups = ps.tile([128, 512], F32, tag="ps", name="wups")
        for i in range(12):
            nc.tensor.matmul(
                wups[:], wz[:, 0:128], wz[:], start=(i == 0), stop=(i == 11),
            )

        # ---- DMA issue order = transfer order: strictly by first use ----
        wk_t = wp.tile([128, NDC, 128], BF16, tag="wk")
        wv_t = wp.tile([128, NDC, 128], BF16, tag="wv")

        def load_x(cq, parts):
            x_q = xp.tile([128, NDC, 512], BF16, tag="x", name=f"x{cq}")
            dg = 0
            for w_dg in parts:
                nc.sync.dma_start(
                    x_q[:, dg : dg + w_dg, :],
                    xt_d[
                        :,
                        cq * NDC * 512 + dg * 512 : cq * NDC * 512
                        + (dg + w_dg) * 512,
                    ],
                )
                dg += w_dg
            return x_q

        # front DMA order = first-use order. Few, mid-sized issues: each
        # dma_start costs 0.6-1.2us of serial Sync time, and concurrent
        # transfers share bandwidth round-robin, so a long tail of small
        # parts delays everything behind it (measured x0 landing at 20us)
        # transfers share bandwidth equally per outstanding dma_start, so the
        # split also acts as a priority: x0 keeps 2-3 queues against wq's one
        nc.sync.dma_start(wk_t[:], wk_d[:, :])
        x_q0 = xp.tile([128, NDC, 512], BF16, tag="x", name="x0")
        wq_t = [
            wp.tile([128, NDC, 256], BF16, tag=f"wq{i}", name=f"wq{i}")
            for i in range(2)
        ]
        nc.sync.dma_start(x_q0[:, 0:2, :], xt_d[:, 0:1024])
        nc.sync.dma_start(x_q0[:, 2:8, :], xt_d[:, 1024:4096])
        nc.sync.dma_start(x_q0[:, 8:16, :], xt_d[:, 4096:8192])
        x_tiles = {0: x_q0}

        fmp = cp.tile([128, S], BF16, tag="fmp")  # fm rows 0:64, fp 64:128
        msk = cp.tile([128, 1024], BF16, tag="msk")  # [tail 0:512 | diag 512:1024]
        idn = cp.tile([128, 128], BF16, tag="idn")
        wo_t = None
        pending = None  # (pv tiles, qc) awaiting normalize/transpose/O-proj
        window = []  # trailing (pv, kb, p_sb, masked) PV groups, oldest first

        def pv_group(pv, kb, p_sb, is_stop):
            for h in range(HPC):
                nc.tensor.matmul(
                    pv[h // 2][:, (h % 2) * 132 : (h % 2) * 132 + 129],
                    p_sb[:, h * 128 : (h + 1) * 128],
                    v_sb[:, kb, 0:129],
                    start=False,
                    stop=is_stop,
                )

        def fire(idx):
            pv, kb, p_sb, masked = window.pop(idx)
            is_stop = not any(w[0] is pv for w in window)
            pv_group(pv, kb, p_sb, is_stop)

        def fire_one():
            # prefer the oldest UNMASKED group: masked p tiles wait on the
            # GpSimd mask multiply (~1.2us), so they fire as late as
            # possible — accumulation order into the PV bank is free
            for j, w in enumerate(window):
                if not w[3]:
                    fire(j)
                    return
            fire(0)

        def flush_to(pv_tiles):
            # a finish reads its PV accumulators: every trailing group that
            # targets them must be emitted first (masked ones last)
            while True:
                idxs = [j for j, w in enumerate(window) if w[0] is pv_tiles]
                if not idxs:
                    return
                unm = [j for j in idxs if not window[j][3]]
                fire(unm[0] if unm else idxs[0])

        def finish_norm(st, h):
            # head h's normalize on the vector engine
            pv = st["pv"]
            c0 = (h % 2) * 132
            if h % 2 == 0:
                # both heads' softmax denominators sit at cols 128/260 of the
                # shared bank: one strided reciprocal covers the pair
                rc = rp.tile([128, 2], F32, tag="rc")
                st["rc"] = rc
                nc.vector.reciprocal(rc[:], pv[h // 2][:, 128:261:132])
            rc = st["rc"]
            a_sb = ap_.tile([128, 128], BF16, tag="a")
            st["a"][h] = a_sb
            nc.vector.tensor_scalar_mul(
                a_sb[:], pv[h // 2][:, c0 : c0 + 128], rc[:, h % 2 : h % 2 + 1]
            )

        def finish_trans(st, h):
            # head h's transpose, one step AFTER its norm: the PE-side
            # transpose otherwise waits in-step on the vector norm chain
            # (measured ~670ns stall per head)
            if st["at"] is None:
                st["at"] = atp.tile([128, HPC, 128], BF16, tag="at", name="at_sb")
            at_sb = st["at"]
            # transpose scratch comes from the ps pool (idle until the O-proj
            # step), NOT sps: an sps tile would couple the finish chain into
            # the score-pair banks via WAR deps
            tps = ps.tile([128, 512], F32, tag="ps", name=f"tp{h}")
            nc.tensor.matmul(tps[:, 0:128], st["a"][h], idn, start=True, stop=True)
            # all 4 copies on vector: scalar is the busier engine during
            # attention (one 690ns exp per pair) and queued copies there
            # delay exps, which shows up as score-matmul stalls
            nc.vector.tensor_copy(at_sb[:, h, :], tps[:, 0:128])

        def finish_tail(st):
            at_sb = st["at"]
            qb = st["qc"] * 128
            last = st["qc"] == NQC - 1
            o_sb = op_.tile([128, 2048], BF16, tag="o")
            for dn in range(4):
                ops = ps.tile([128, 512], F32, tag="ps")
                for f in range(HPC):
                    nc.tensor.matmul(
                        ops[:],
                        at_sb[:, f, :],
                        wo_t[:, f, dn * 512 : (dn + 1) * 512],
                        start=(f == 0),
                        stop=(f == HPC - 1),
                    )
                if last and dn % 2 == 1:
                    # final q-block only: copies drain two-wide across
                    # vector+scalar (no later exps to collide with), so the
                    # four store DMAs issue ~700ns sooner each off the tail
                    nc.scalar.copy(o_sb[:, dn * 512 : (dn + 1) * 512], ops[:])
                else:
                    nc.vector.tensor_copy(
                        o_sb[:, dn * 512 : (dn + 1) * 512], ops[:]
                    )
                if last:
                    # final q-block: store each quarter as soon as its copy
                    # lands so the DMA overlaps the remaining O-proj matmuls
                    nc.sync.dma_start(
                        out_d[qb : qb + 128, dn * 512 : (dn + 1) * 512],
                        o_sb[:, dn * 512 : (dn + 1) * 512],
                    )
                elif dn == 1:
                    nc.sync.dma_start(
                        out_d[qb : qb + 128, 0:1024], o_sb[:, 0:1024]
                    )
            # two DMAs per q-block: fused enough to keep Sync sequencing
            # cheap (~600ns per dma_start), split so the final q-block's
            # store overlaps its second O-proj half
            if not last:
                nc.sync.dma_start(
                    out_d[qb : qb + 128, 1024:2048], o_sb[:, 1024:2048]
                )

        def finish_step(st):
            # advance the staged finish by one unit; True when fully done.
            # Stages: 0: flush+norm0, 1-3: norm h / transpose h-1, 4:
            # transpose 3, 5: O-proj tail
            s = st["step"]
            if s == 0:
                flush_to(st["pv"])
                finish_norm(st, 0)
            elif s <= 3:
                finish_norm(st, s)
                finish_trans(st, s - 1)
            elif s == 4:
                finish_trans(st, 3)
            else:
                finish_tail(st)
            st["step"] += 1
            return st["step"] > HPC + 1

        def emit_finish(st):
            while not finish_step(st):
                pass

        for cq in range(4):
            # ---- projections for column-quarter cq ----
            x_q = x_tiles.pop(cq)
            if cq == 0:
                # remaining weights/constants in first-use order; wv/wo land
                # during this quarter's Q-proj
                nc.sync.dma_start(wq_t[0][:], wq_d[:, 0:4096])
                nc.sync.dma_start(fmp[:, 0:512], fmp_d[:, 0:512])
                nc.sync.dma_start(wq_t[1][:], wq_d[:, 4096:8192])
                nc.sync.dma_start(wv_t[:], wv_d[:, :])
                nc.sync.dma_start(msk[:], msk_d[:, :])
                nc.sync.dma_start(idn[:], idn_d[:, :])
                nc.vector.memset(v_sb[:, :, 128:129], 1.0)
                wo_t = wp.tile([128, HPC, 2048], BF16, tag="wo")
                for i in range(2):
                    nc.sync.dma_start(
                        wo_t[:, i * 2 : (i + 1) * 2, :],
                        wo_d[:, i * 4096 : (i + 1) * 4096],
                    )
                nc.sync.dma_start(fmp[:, 512:S], fmp_d[:, 512:S])
            cs = slice(cq * 512, (cq + 1) * 512)
            fmc, fpc = fmp[0:64, cs], fmp[64:128, cs]
            blk = slice(cq * 4, (cq + 1) * 4)

            def q_rope(h, qps):
                hc = slice(h * 128, (h + 1) * 128)
                src = qps[0:64, :].rearrange("p (b q) -> p b q", b=4)
                nc.vector.tensor_mul(
                    q1[0:64, blk, hc], src, fmc.rearrange("p (b q) -> p b q", b=4)
                )
                nc.vector.tensor_mul(
                    q1[64:128, blk, hc], src, fpc.rearrange("p (b q) -> p b q", b=4)
                )
                nc.scalar.copy(
                    q2[0:64, blk, hc],
                    qps[64:128, :].rearrange("p (b q) -> p b q", b=4),
                )
                nc.scalar.copy(
                    q2[64:128, blk, hc],
                    qps[64:128, :].rearrange("p (b q) -> p b q", b=4),
                )

            # K projection + rope
            kps = ps.tile([128, 512], F32, tag="ps")
            for dc in range(NDC):
                nc.tensor.matmul(
                    kps[:],
                    wk_t[:, dc, :],
                    x_q[:, dc, :],
                    start=(dc == 0),
                    stop=(dc == NDC - 1),
                )
            nc.vector.tensor_mul(k1[0:64, cs], kps[0:64, :], fmc)
            nc.vector.tensor_mul(k1[64:128, cs], kps[0:64, :], fpc)
            nc.scalar.copy(k2[0:64, cs], kps[64:128, :])
            nc.scalar.copy(k2[64:128, cs], kps[64:128, :])

            # Q projections + rope, packed layout [feat, qblock, h*128+q]
            for h in range(HPC):
                qps = ps.tile([128, 512], F32, tag="ps")
                for dc in range(NDC):
                    nc.tensor.matmul(
                        qps[:],
                        wq_t[h // 2][:, dc, (h % 2) * 128 : (h % 2 + 1) * 128],
                        x_q[:, dc, :],
                        start=(dc == 0),
                        stop=(dc == NDC - 1),
                    )
                q_rope(h, qps)

            # V projection -> v_sb [kpos, feat] (first consumed mid-way into
            # this quarter's attention, so it sits after Q to shorten the
            # rope -> first-scores critical chain)
            vps = ps.tile([128, 512], F32, tag="ps")
            for kb4 in range(4):
                for dc in range(NDC):
                    nc.tensor.matmul(
                        vps[:, kb4 * 128 : (kb4 + 1) * 128],
                        x_q[:, dc, kb4 * 128 : (kb4 + 1) * 128],
                        wv_t[:, dc, :],
                        start=(dc == 0),
                        stop=(dc == NDC - 1),
                    )
            nc.vector.tensor_copy(
                v_sb[:, cq * 4 : (cq + 1) * 4, 0:128],
                vps[:].rearrange("p (b q) -> p b q", b=4),
            )

            # prefetch next x chunk during this quarter's attention
            if cq + 1 < 4:
                x_tiles[cq + 1] = load_x(cq + 1, [8, 8])

            # finish the previous quarter's last q-block now: its transposes
            # + O-projection give the PE work while the rope tail (vector/
            # scalar) of this quarter completes
            if pending is not None:
                emit_finish(pending)
                pending = None

            # ---- attention for q-blocks of this quarter ----
            # quarter 0 runs [3,2,1,0]: the ramp q-blocks (1-2 score pairs)
            # can't hide their exp->mask->PV chains, so they sit against the
            # quarter boundary where cq1's projections cover them
            qcs = [3, 2, 1, 0] if cq == 0 else range(cq * 4, (cq + 1) * 4)
            for qc in qcs:
                kbs = _kblocks(qc)
                nkb = len(kbs)
                pv = [
                    pvs.tile([128, 264], F32, tag="pv", name=f"pv{qc}_{i}")
                    for i in range(2)
                ]
                # Two heads accumulate in one bank: a start=True matmul would
                # clear the co-resident head's has_written bits mid-group, so
                # zero the bank and accumulate with start=False throughout
                # (add-where-set on zeros / overwrite-where-clear both work).
                for t in pv:
                    nc.vector.memset(t[:], 0.0)

                # masked k-blocks (diagonal, window tail) first: their extra
                # post-exp mask multiply sits on the exp->PV chain, so keep
                # them off the q-block tail where that chain is exposed
                kbs_proc = [kbs[-1]] + ([kbs[0]] if nkb > 1 else []) + kbs[1:-1]
                for mi, kb in enumerate(kbs_proc):
                    sp = sps.tile([128, 512], F32, tag="s")
                    lo = kb * 128
                    nc.tensor.matmul(
                        sp[:], k1[:, lo : lo + 128], q1[:, qc, :],
                        start=True, stop=False,
                    )
                    nc.tensor.matmul(
                        sp[:], k2[:, lo : lo + 128], q2[:, qc, :],
                        start=False, stop=True,
                    )
                    p_sb = pp.tile([128, 512], BF16, tag="p")
                    nc.scalar.activation(p_sb[:], sp[:], EXP)
                    # sliding-window mask: 0/1 multiply after the exp, on the
                    # otherwise-idle GpSimd engine (SBUF-only op)
                    masked = kb == qc or kb == qc - 8
                    if kb == qc:
                        nc.gpsimd.tensor_mul(p_sb[:], p_sb[:], msk[:, 512:1024])
                    if kb == qc - 8:
                        nc.gpsimd.tensor_mul(p_sb[:], p_sb[:], msk[:, 0:512])
                    # the PV group trails by 4 k-blocks, carried across
                    # q-block boundaries so score matmuls always cover the
                    # exp latency
                    window.append((pv, kb, p_sb, masked))
                    # the previous q-block's finish advances one stage per
                    # k-block: each transpose trails the vector recip/norm
                    # chain with a score pair of PE cover in between
                    if pending is not None and mi >= 1:
                        if finish_step(pending):
                            pending = None
                    while len(window) > 4:
                        fire_one()
                while pending is not None:
                    if finish_step(pending):
                        pending = None
                pending = {
                    "pv": pv, "qc": qc, "at": None, "rc": None, "a": {},
                    "step": 0,
                }

        emit_finish(pending)

    nc.compile()
    return nc


def _prep_core(inputs, c):
    x = inputs["x"]
    cos, sin = np.asarray(inputs["cos"]), np.asarray(inputs["sin"])
    mask = np.asarray(inputs["mask"])
    wq = np.asarray(inputs["wq"], dtype=np.float32)
    wk = np.asarray(inputs["wk"], dtype=np.float32)
    wv = np.asarray(inputs["wv"], dtype=np.float32)
    wo = np.asarray(inputs["wo"], dtype=np.float32)
    bf = ml_dtypes.bfloat16
    b, g = c // 4, c % 4

    # x[b] transposed -> [128p, cq, dc, 512]
    xt = np.asarray(x[b], dtype=np.float32).T  # [dim, S]
    xt = xt.reshape(NDC, 128, 4, 512).transpose(1, 2, 0, 3)
    xt = np.ascontiguousarray(xt).reshape(128, 4 * NDC * 512).astype(bf)

    # wq slice for heads 4g..4g+3 (SCALE folded). The imag half-columns are
    # additionally halved: the kernel duplicates q2/k2 across both partition
    # halves, doubling the imag contraction.
    wqs = (wq[:, g * 512 : (g + 1) * 512] * SCALE).reshape(DIM, HPC, 128).copy()
    wqs[:, :, 64:128] *= 0.5
    wqs = wqs.reshape(NDC, 128, 2, 256)
    wqs = np.ascontiguousarray(wqs.transpose(1, 2, 0, 3)).reshape(128, 2 * NDC * 256)
    # wk / wv slices for kv head g: [p, dc, 128]
    wks = wk[:, g * 128 : (g + 1) * 128].reshape(NDC, 128, 128).transpose(1, 0, 2)
    wks = np.ascontiguousarray(wks).reshape(128, NDC * 128)
    wvs = wv[:, g * 128 : (g + 1) * 128].reshape(NDC, 128, 128).transpose(1, 0, 2)
    wvs = np.ascontiguousarray(wvs).reshape(128, NDC * 128)
    # wo rows for this core's heads: [p, h, 2048] tiles
    wos = wo[g * 512 : (g + 1) * 512].reshape(HPC, 128, 2048).transpose(1, 0, 2)
    wos = np.ascontiguousarray(wos).reshape(128, HPC * 2048)

    # fm rows 0:64, fp rows 64:128 (one tensor -> one DMA)
    fmp = np.concatenate([(cos - sin).T, (cos + sin).T], axis=0)
    fmp = np.ascontiguousarray(fmp, dtype=np.float32).astype(bf)
    # 0/1 keep-masks, transposed for the S^T layout, tiled across the 4
    # packed heads: [tail block | diagonal block]
    tail01 = (mask[WINDOW : WINDOW + 128, 0:128].T == 0.0).astype(np.float32)
    diag01 = (mask[0:128, 0:128].T == 0.0).astype(np.float32)
    msk = np.concatenate([np.tile(tail01, (1, 4)), np.tile(diag01, (1, 4))], axis=1)
    msk = np.ascontiguousarray(msk).astype(bf)
    idn = np.ascontiguousarray(np.eye(128, dtype=np.float32)).astype(bf)

    return {
        "xt": xt, "wq": wqs.astype(bf), "wk": wks.astype(bf), "wv": wvs.astype(bf),
        "wo": wos.astype(bf), "fmp": fmp, "msk": msk, "idn": idn,
    }


def kernel(**inputs) -> np.ndarray:
    if "nc" not in _cache:
        _cache["nc"] = _build()
    nc = _cache["nc"]
    in_maps = [_prep_core(inputs, c) for c in range(8)]
    res = run_bass_kernel_spmd(nc, in_maps, core_ids=list(range(8)))
    out = np.zeros((B, S, DIM), dtype=np.float32)
    for c in range(8):
        out[c // 4] += np.asarray(res.results[c]["out"], dtype=np.float32)
    return out
